# revision 1
# baseline (speedup 1.0000x reference)
"""SchNet-style GNN message passing on 8 Trainium2 NeuronCores.

Strategy (pure data parallel over the graph batch, per sharding hint):
- Nodes are split into 8 contiguous, graph-aligned ranges (batch is sorted).
- Each edge is owned by the core owning its dst node; per-core edges are
  sorted by dst and tiled into 128-message tiles that each fit a 128-node
  "window" of the destination range.
- Per layer: every core computes x = h_own @ cf_w1 for its own nodes,
  AllGathers the bf16 x-table (row layout) across cores, bulk-gathers
  x[src] rows with dma_gather (int16 indices => the table is addressed in
  a lo half [<32768] and a hi half; edges are processed in two phases),
  runs the filter MLP on-chip, multiplies, and scatter-adds messages via
  one-hot indicator matmuls on the PE (indicators are host-built, with the
  cosine cutoff C folded in).
- The per-channel filter bias b2 (and the softplus -log2 shift) is applied
  algebraically: agg_true = scatter(xg*Wf_raw) + b2_eff ⊙ scatter(xg), and
  the second term is folded into the cf_w2 matmul with a pre-scaled copy
  of cf_w2.
- Readout (segment-sum over graphs + MLP) runs locally per core.
"""

import numpy as np
import ml_dtypes

import concourse.bacc as bacc
import concourse.bass as bass
import concourse.tile as tile
from concourse import mybir
from concourse import bass_utils
from concourse.library_config import mlp as _mlp_lib

BF16 = ml_dtypes.bfloat16
P = 128
H = 128
NGAUSS = 50
L = 3
CUTOFF = 10.0
LOG2 = float(np.log(2.0))
NC = 8
SPLIT = 32768
GROUP = 4          # message tiles per filter/gather group (512 edges)
F32 = mybir.dt.float32
BF = mybir.dt.bfloat16
I16 = mybir.dt.int16
I32 = mybir.dt.int32


def _wrap16(vals, ncols):
    """dma_gather index layout: [16, n/16] wrapped, replicated to 128 partitions."""
    a = np.zeros((16, ncols), np.int16)
    n = len(vals)
    a[np.arange(n) % 16, np.arange(n) // 16] = vals.astype(np.int16)
    return np.tile(a, (8, 1))


def _host_prep(z, edge_src, edge_dst, batch, G, edge_weight, edge_attr):
    N = z.shape[0]
    E = edge_src.shape[0]

    counts = np.bincount(batch, minlength=G)
    cum = np.concatenate([[0], np.cumsum(counts)])  # node start of each graph
    # graph-aligned node boundaries, balanced by node count
    g_bound = np.zeros(NC + 1, np.int64)
    g_bound[NC] = G
    for c in range(1, NC):
        g_bound[c] = np.searchsorted(cum, c * N / NC)
    n_bound = cum[g_bound]

    n_own = np.diff(n_bound)
    NP = int(np.ceil(n_own.max() / P) * P)          # padded nodes per core
    W_CNT = NP // P
    Gmax = int(np.diff(g_bound).max())

    owner = np.searchsorted(n_bound, np.arange(N), side="right") - 1
    local = np.arange(N) - n_bound[owner]
    table_row = owner * NP + local                   # row in allgathered x table

    C_all = (0.5 * (np.cos(edge_weight * np.pi / CUTOFF) + 1.0)).astype(np.float32)

    e_owner = owner[edge_dst]
    src_row = table_row[edge_src]
    lo_mask = src_row < SPLIT

    # per (core, phase, window) edge counts -> uniform tiles per window
    T_pw = [0, 0]
    per_core = []
    for c in range(NC):
        sel = np.nonzero(e_owner == c)[0]
        ldst = local[edge_dst[sel]]
        order = np.argsort(ldst, kind="stable")
        sel = sel[order]
        ldst = ldst[order]
        win = ldst // P
        lo = lo_mask[sel]
        per_core.append((sel, ldst, win, lo))
        for ph in range(2):
            m = lo if ph == 0 else ~lo
            cnt = np.bincount(win[m], minlength=W_CNT)
            T_pw[ph] = max(T_pw[ph], int(np.ceil(cnt.max() / P)))
    T_pw = [max(t, 1) for t in T_pw]
    NT_A = W_CNT * T_pw[0]
    NT_B = W_CNT * T_pw[1]
    # pad each phase's tile count to a multiple of GROUP
    NT_A = int(np.ceil(NT_A / GROUP) * GROUP)
    NT_B = int(np.ceil(NT_B / GROUP) * GROUP)
    NG_A = NT_A // GROUP
    NG_B = NT_B // GROUP
    N_T = NT_A + NT_B

    meta = dict(NP=NP, W_CNT=W_CNT, Gmax=Gmax, T_pw=T_pw, NT_A=NT_A, NT_B=NT_B,
                NG_A=NG_A, NG_B=NG_B, N_T=N_T, n_bound=n_bound, g_bound=g_bound)

    per_core_arrays = []
    for c in range(NC):
        sel, ldst, win, lo = per_core[c]
        ea_full = np.zeros((N_T * P, NGAUSS), np.float32)
        ind_full = np.zeros((N_T * P, P), np.float32)
        src_full = np.zeros(N_T * P, np.int64)

        for ph in range(2):
            m = lo if ph == 0 else ~lo
            e_idx = sel[m]
            w_ph = win[m]
            l_ph = ldst[m]
            tpw = T_pw[ph]
            base = 0 if ph == 0 else NT_A
            # position within window (edges already window-sorted)
            cnt = np.bincount(w_ph, minlength=W_CNT)
            startw = np.concatenate([[0], np.cumsum(cnt)])
            k = np.arange(len(e_idx)) - startw[w_ph]
            slot = (base + w_ph * tpw + k // P) * P + (k % P)
            ea_full[slot] = edge_attr[e_idx]
            ind_full[slot, l_ph - w_ph * P] = C_all[e_idx]
            sr = src_row[e_idx]
            src_full[slot] = np.where(m[m], sr - (0 if ph == 0 else SPLIT), 0)

        # group-major layouts
        # ea_g: [NGRP, 50, GROUP*128] bf16 (transposed per group)
        ea_g = (ea_full.reshape(N_T // GROUP, GROUP * P, NGAUSS)
                .transpose(0, 2, 1).astype(BF16))
        # ind_g: [NGRP, 128, GROUP*128] bf16 : [p, tloc*128+col] = ind[tile, p, col]
        ind_g = (ind_full.reshape(N_T // GROUP, GROUP, P, P)
                 .transpose(0, 2, 1, 3).reshape(N_T // GROUP, P, GROUP * P)
                 .astype(BF16))
        # gather idx wrapped per group: [128, NGRP*GROUP*128/16]
        npg = GROUP * P // 16
        idx_w = np.zeros((P, (N_T // GROUP) * npg), np.int16)
        for g in range(N_T // GROUP):
            idx_w[:, g * npg:(g + 1) * npg] = _wrap16(
                src_full[g * GROUP * P:(g + 1) * GROUP * P], npg)

        # node init: z indices [128, W_CNT]
        ns, ne = n_bound[c], n_bound[c + 1]
        zq = np.zeros(NP, np.int64)
        zq[: ne - ns] = z[ns:ne]
        zq = zq.reshape(W_CNT, P).T.astype(np.int32).copy()

        # graph indicator [W_CNT, 128, Gmax]
        gs, ge = g_bound[c], g_bound[c + 1]
        gi = np.zeros((NP, Gmax), np.float32)
        gl = batch[ns:ne] - gs
        gi[np.arange(ne - ns), gl] = 1.0
        gind = gi.reshape(W_CNT, P, Gmax)

        per_core_arrays.append(dict(ea_g=ea_g, ind_g=ind_g, idx_w=idx_w,
                                    zq=zq, gind=gind))
    return meta, per_core_arrays


def _build(meta, weights):
    NP, W_CNT, Gmax = meta["NP"], meta["W_CNT"], meta["Gmax"]
    NT_A, NT_B, N_T = meta["NT_A"], meta["NT_B"], meta["N_T"]
    NGRP = N_T // GROUP
    npg = GROUP * P // 16

    nc = bacc.Bacc("TRN2", target_bir_lowering=False, debug=False,
                   enable_asserts=False, num_devices=NC)

    d_ea = nc.dram_tensor("ea_g", [NGRP, NGAUSS, GROUP * P], BF, kind="ExternalInput")
    d_ind = nc.dram_tensor("ind_g", [NGRP, P, GROUP * P], BF, kind="ExternalInput")
    d_idx = nc.dram_tensor("idx_w", [P, NGRP * npg], I16, kind="ExternalInput")
    d_zq = nc.dram_tensor("zq", [P, W_CNT], I32, kind="ExternalInput")
    d_gind = nc.dram_tensor("gind", [W_CNT, P, Gmax], F32, kind="ExternalInput")
    d_emb = nc.dram_tensor("emb", [120, H], F32, kind="ExternalInput")
    d_w1 = nc.dram_tensor("w1", [L, NGAUSS, H], BF, kind="ExternalInput")
    d_b1 = nc.dram_tensor("b1", [L, H, 1], F32, kind="ExternalInput")
    d_w2 = nc.dram_tensor("w2", [L, H, H], BF, kind="ExternalInput")
    d_cfw1 = nc.dram_tensor("cfw1", [L, H, H], F32, kind="ExternalInput")
    d_cfw2 = nc.dram_tensor("cfw2", [L, H, H], F32, kind="ExternalInput")
    d_cfw2b = nc.dram_tensor("cfw2b", [L, H, H], F32, kind="ExternalInput")
    d_cfb2 = nc.dram_tensor("cfb2", [L, H, 1], F32, kind="ExternalInput")
    d_linw = nc.dram_tensor("linw", [L, H, H], F32, kind="ExternalInput")
    d_linb = nc.dram_tensor("linb", [L, H, 1], F32, kind="ExternalInput")
    d_row1 = nc.dram_tensor("row1", [H, 5 * H], F32, kind="ExternalInput")
    d_rob1 = nc.dram_tensor("rob1", [P, 5 * H], F32, kind="ExternalInput")
    d_row2 = nc.dram_tensor("row2", [5, H, H], F32, kind="ExternalInput")
    d_rob2 = nc.dram_tensor("rob2", [H, 1], F32, kind="ExternalInput")
    d_row3 = nc.dram_tensor("row3", [H, 1], F32, kind="ExternalInput")
    d_ident = nc.dram_tensor("ident", [P, P], F32, kind="ExternalInput")
    d_out = nc.dram_tensor("out", [Gmax, 1], F32, kind="ExternalOutput")
    b3_eff = weights["b3_eff"]

    EXP = mybir.ActivationFunctionType.Exp
    LN = mybir.ActivationFunctionType.Ln
    RELU = mybir.ActivationFunctionType.Relu
    ABS = mybir.ActivationFunctionType.Abs
    MUL = mybir.AluOpType.mult
    ADD = mybir.AluOpType.add

    with tile.TileContext(nc) as tc:
        with tc.tile_pool(name="const", bufs=1) as cst, \
             tc.tile_pool(name="big", bufs=1) as big, \
             tc.tile_pool(name="dram", bufs=1, space="DRAM") as drp:

            nc.gpsimd.load_library(_mlp_lib)

            # resident tiles
            idx_t = cst.tile([P, NGRP * npg], I16)
            nc.sync.dma_start(idx_t[:], d_idx[:])
            zq_t = cst.tile([P, W_CNT], I32)
            nc.sync.dma_start(zq_t[:], d_zq[:])
            ident_t = cst.tile([P, P], F32)
            nc.sync.dma_start(ident_t[:], d_ident[:])
            w1_t = [cst.tile([NGAUSS, H], BF, tag=f"w1_{l}", name=f"w1_{l}") for l in range(L)]
            b1_t = [cst.tile([H, 1], F32, tag=f"b1_{l}", name=f"b1_{l}") for l in range(L)]
            w2_t = [cst.tile([H, H], BF, tag=f"w2_{l}", name=f"w2_{l}") for l in range(L)]
            cfw1_t = [cst.tile([H, H], F32, tag=f"cfw1_{l}", name=f"cfw1_{l}") for l in range(L)]
            cfw2_t = [cst.tile([H, H], F32, tag=f"cfw2_{l}", name=f"cfw2_{l}") for l in range(L)]
            cfw2b_t = [cst.tile([H, H], F32, tag=f"cfw2b_{l}", name=f"cfw2b_{l}") for l in range(L)]
            cfb2_t = [cst.tile([H, 1], F32, tag=f"cfb2_{l}", name=f"cfb2_{l}") for l in range(L)]
            linw_t = [cst.tile([H, H], F32, tag=f"linw_{l}", name=f"linw_{l}") for l in range(L)]
            linb_t = [cst.tile([H, 1], F32, tag=f"linb_{l}", name=f"linb_{l}") for l in range(L)]
            for l in range(L):
                nc.sync.dma_start(w1_t[l][:], d_w1[l])
                nc.sync.dma_start(b1_t[l][:], d_b1[l])
                nc.sync.dma_start(w2_t[l][:], d_w2[l])
                nc.sync.dma_start(cfw1_t[l][:], d_cfw1[l])
                nc.sync.dma_start(cfw2_t[l][:], d_cfw2[l])
                nc.sync.dma_start(cfw2b_t[l][:], d_cfw2b[l])
                nc.sync.dma_start(cfb2_t[l][:], d_cfb2[l])
                nc.sync.dma_start(linw_t[l][:], d_linw[l])
                nc.sync.dma_start(linb_t[l][:], d_linb[l])
            row1_t = cst.tile([H, 5 * H], F32)
            nc.sync.dma_start(row1_t[:], d_row1[:])
            rob1_t = cst.tile([P, 5 * H], F32)
            nc.sync.dma_start(rob1_t[:], d_rob1[:])
            row2_t = [cst.tile([H, H], F32, tag=f"row2_{i}", name=f"row2_{i}") for i in range(5)]
            for i in range(5):
                nc.sync.dma_start(row2_t[i][:], d_row2[i])
            rob2_t = cst.tile([H, 1], F32)
            nc.sync.dma_start(rob2_t[:], d_rob2[:])
            row3_t = cst.tile([H, 1], F32)
            nc.sync.dma_start(row3_t[:], d_row3[:])

            hT = big.tile([P, NP], F32)          # h_own^T
            aggA = big.tile([P, W_CNT * 256], F32)  # [aggT_w | aggxT_w] interleaved
            aggB = big.tile([P, W_CNT * 256], F32)
            x_st = big.tile([P, W_CNT * H], BF)  # x_own rows staging

            x_own_ds = [drp.tile([NP, H], BF, name=f"x_own_{l}", tag=f"x_own_{l}")
                        for l in range(L)]
            x_full_ds = [drp.tile([NC * NP, H], BF, addr_space="Shared",
                                  name=f"x_full_{l}", tag=f"x_full_{l}")
                         for l in range(L)]

            with tc.tile_pool(name="pp", bufs=4, space="PSUM") as pp, \
                 tc.tile_pool(name="psca", bufs=2, space="PSUM") as psca, \
                 tc.tile_pool(name="pscx", bufs=2, space="PSUM") as pscx, \
                 tc.tile_pool(name="wk", bufs=3) as wk:

                # ---- h0 = relu(emb[z])^T ----
                for j in range(W_CNT):
                    rows = wk.tile([P, H], F32, tag="h0rows")
                    nc.gpsimd.indirect_dma_start(
                        out=rows[:], out_offset=None, in_=d_emb[:],
                        in_offset=bass.IndirectOffsetOnAxis(ap=zq_t[:, j:j + 1], axis=0))
                    nc.vector.tensor_scalar_max(rows[:], rows[:], 0.0)
                    pt = pp.tile([P, 512], F32, tag="pp", name="pxt")[:, 0:128]
                    nc.tensor.transpose(pt[:], rows[:], ident_t[:])
                    nc.vector.tensor_copy(hT[:, j * P:(j + 1) * P], pt[:])

                for l in range(L):
                    x_own_d = x_own_ds[l]
                    x_full_d = x_full_ds[l]
                    # ---- x_own = h_own @ cf_w1[l]  (f32 matmul, rows, -> bf16) ----
                    for j0 in range(0, W_CNT, 4):
                        jn = min(4, W_CNT - j0)
                        pxt = pp.tile([P, 512], F32, tag="pp", name="pxt2")
                        for jj in range(jn):
                            nc.tensor.matmul(
                                pxt[:, jj * H:(jj + 1) * H],
                                lhsT=hT[:, (j0 + jj) * P:(j0 + jj + 1) * P],
                                rhs=cfw1_t[l][:], start=True, stop=True)
                        nc.vector.tensor_copy(
                            x_st[:, j0 * H:(j0 + jn) * H], pxt[:, :jn * H])
                    nc.sync.dma_start(
                        x_own_d[:].rearrange("(w p) h -> p w h", p=P),
                        x_st[:].rearrange("p (w h) -> p w h", h=H))
                    nc.gpsimd.collective_compute(
                        "AllGather", mybir.AluOpType.bypass,
                        replica_groups=[list(range(NC))],
                        ins=[x_own_d.opt()], outs=[x_full_d.opt()])

                    # ---- edge phases ----
                    for ph in range(2):
                        ng = (NT_A if ph == 0 else NT_B) // GROUP
                        g_off = 0 if ph == 0 else NT_A // GROUP
                        t_off = 0 if ph == 0 else NT_A
                        tpw = meta["T_pw"][ph]
                        tbl = x_full_d[:SPLIT, :] if ph == 0 else x_full_d[SPLIT:, :]
                        agg = aggA if ph == 0 else aggB
                        cur_ps = None
                        for g in range(ng):
                            gg = g_off + g
                            ea = wk.tile([NGAUSS, GROUP * P], BF, tag="ea")
                            nc.sync.dma_start(ea[:], d_ea[gg])
                            ind = wk.tile([P, GROUP * P], BF, tag="ind")
                            nc.sync.dma_start(ind[:], d_ind[gg])
                            xg = wk.tile([P, GROUP * H], BF, tag="xg")
                            nc.gpsimd.dma_gather(
                                xg[:].rearrange("p (k h) -> p k h", h=H),
                                tbl, idx_t[:, gg * npg:(gg + 1) * npg],
                                GROUP * P, GROUP * P, H)
                            p1 = pp.tile([P, GROUP * P], F32, tag="pp", name="p1t")
                            nc.tensor.matmul(p1[:], lhsT=w1_t[l][:], rhs=ea[:],
                                             start=True, stop=True)
                            e1 = wk.tile([P, GROUP * P], F32, tag="e1")
                            nc.scalar.activation(e1[:], p1[:], EXP,
                                                 bias=b1_t[l][:, 0:1], scale=1.0)
                            ss = wk.tile([P, GROUP * P], BF, tag="ss")
                            nc.scalar.activation(ss[:], e1[:], LN,
                                                 bias=1.0, scale=1.0)
                            p2 = pp.tile([P, GROUP * P], F32, tag="pp", name="p2t")
                            for tt in range(GROUP):
                                nc.tensor.matmul(
                                    p2[:, tt * H:(tt + 1) * H],
                                    lhsT=ss[:, tt * P:(tt + 1) * P],
                                    rhs=w2_t[l][:], start=True, stop=True)
                            msg = wk.tile([P, GROUP * H], BF, tag="msg")
                            nc.vector.tensor_tensor(msg[:], p2[:], xg[:], op=MUL)
                            for tt in range(GROUP):
                                t = t_off + g * GROUP + tt
                                tl = t - t_off
                                if tl >= W_CNT * tpw:
                                    break
                                w = tl // tpw
                                first = (tl % tpw == 0)
                                last = (tl % tpw == tpw - 1)
                                if first:
                                    cur_a = psca.tile([P, 128], F32, tag="sa", name="sa")
                                    cur_x = pscx.tile([P, 128], F32, tag="sx", name="sx")
                                nc.tensor.matmul(
                                    cur_a[:],
                                    lhsT=msg[:, tt * H:(tt + 1) * H],
                                    rhs=ind[:, tt * P:(tt + 1) * P],
                                    start=first, stop=last)
                                nc.tensor.matmul(
                                    cur_x[:],
                                    lhsT=xg[:, tt * H:(tt + 1) * H],
                                    rhs=ind[:, tt * P:(tt + 1) * P],
                                    start=first, stop=last)
                                if last:
                                    nc.vector.tensor_copy(
                                        agg[:, w * 256:w * 256 + 128], cur_a[:])
                                    nc.vector.tensor_copy(
                                        agg[:, w * 256 + 128:w * 256 + 256], cur_x[:])

                    # ---- x2/x3 chain + h update ----
                    for j0 in range(0, W_CNT, 4):
                        jn = min(4, W_CNT - j0)
                        p2x = pp.tile([P, 512], F32, tag="pp", name="pxt3")
                        a4 = aggA[:].rearrange("p (w t c) -> p w t c", t=2, c=128)
                        b4 = aggB[:].rearrange("p (w t c) -> p w t c", t=2, c=128)
                        nc.tensor.matmul(p2x[:, :jn * P], lhsT=cfw2_t[l][:],
                                         rhs=a4[:, j0:j0 + jn, 0, :],
                                         start=True, stop=False)
                        nc.tensor.matmul(p2x[:, :jn * P], lhsT=cfw2b_t[l][:],
                                         rhs=a4[:, j0:j0 + jn, 1, :],
                                         start=False, stop=False)
                        nc.tensor.matmul(p2x[:, :jn * P], lhsT=cfw2_t[l][:],
                                         rhs=b4[:, j0:j0 + jn, 0, :],
                                         start=False, stop=False)
                        nc.tensor.matmul(p2x[:, :jn * P], lhsT=cfw2b_t[l][:],
                                         rhs=b4[:, j0:j0 + jn, 1, :],
                                         start=False, stop=True)
                        e2 = wk.tile([P, 512], F32, tag="e2", bufs=2)
                        nc.scalar.activation(e2[:, :jn * P], p2x[:, :jn * P], EXP,
                                             bias=cfb2_t[l][:, 0:1], scale=1.0)
                        s2 = wk.tile([P, 512], F32, tag="s2", bufs=2)
                        nc.scalar.activation(s2[:, :jn * P], e2[:, :jn * P], LN,
                                             bias=1.0, scale=1.0)
                        p3x = pp.tile([P, 512], F32, tag="pp", name="pxt4")
                        nc.tensor.matmul(p3x[:, :jn * P], lhsT=linw_t[l][:],
                                         rhs=s2[:, :jn * P], start=True, stop=True)
                        tmpu = wk.tile([P, 512], F32, tag="hupd", bufs=2)
                        nc.vector.tensor_scalar(
                            tmpu[:, :jn * P], p3x[:, :jn * P],
                            linb_t[l][:, 0:1], None, op0=ADD)
                        sl = slice(j0 * P, (j0 + jn) * P)
                        nc.vector.tensor_tensor(hT[:, sl], hT[:, sl],
                                                tmpu[:, :jn * P], op=ADD)

            # ---- readout ----
            with tc.tile_pool(name="pro", bufs=1, space="PSUM") as pro, \
                 tc.tile_pool(name="ph2", bufs=2, space="PSUM") as ph2, \
                 tc.tile_pool(name="wk2", bufs=2) as wk2:
                pooledT = pro.tile([P, 5 * 512], F32)   # 5 chunks x [128, Gmax<=128]
                for j in range(W_CNT):
                    ph_a = ph2.tile([P, 512], F32, tag="ro", name="roha")
                    ph_b = ph2.tile([P, 512], F32, tag="ro", name="rohb")[:, 0:128]
                    nc.tensor.matmul(ph_a[:], lhsT=hT[:, j * P:(j + 1) * P],
                                     rhs=row1_t[:, 0:512], start=True, stop=True)
                    nc.tensor.matmul(ph_b[:], lhsT=hT[:, j * P:(j + 1) * P],
                                     rhs=row1_t[:, 512:640], start=True, stop=True)
                    hhf = wk2.tile([P, 5 * H], F32, tag="hhf")
                    nc.vector.tensor_tensor(hhf[:, 0:512], ph_a[:],
                                            rob1_t[:, 0:512], op=ADD)
                    nc.vector.tensor_tensor(hhf[:, 512:640], ph_b[:],
                                            rob1_t[:, 512:640], op=ADD)
                    eh = wk2.tile([P, 5 * H], F32, tag="eh")
                    nc.scalar.activation(eh[:], hhf[:], EXP, bias=0.0, scale=1.0)
                    hh = wk2.tile([P, 5 * H], F32, tag="hh")
                    nc.scalar.activation(hh[:], eh[:], LN, bias=1.0, scale=1.0)
                    nc.vector.tensor_scalar_add(hh[:], hh[:], -LOG2)
                    gi = wk2.tile([P, Gmax], F32, tag="gi")
                    nc.sync.dma_start(gi[:], d_gind[j])
                    for c5 in range(5):
                        nc.tensor.matmul(
                            pooledT[:, c5 * 512:c5 * 512 + Gmax],
                            lhsT=hh[:, c5 * H:(c5 + 1) * H], rhs=gi[:],
                            start=(j == 0), stop=(j == W_CNT - 1))
                plf = wk2.tile([P, 5 * P], F32, tag="plf")
                nc.vector.tensor_copy(
                    plf[:].rearrange("p (c g) -> p c g", g=P),
                    pooledT[:].rearrange("p (c g) -> p c g", g=512)[:, :, 0:P])
                po2 = ph2.tile([P, 512], F32, tag="ro", name="roo2")[:, 0:128]
                for c5 in range(5):
                    nc.tensor.matmul(po2[:, 0:Gmax], lhsT=row2_t[c5][:],
                                     rhs=plf[:, c5 * P:c5 * P + Gmax],
                                     start=(c5 == 0), stop=(c5 == 4))
                ra = wk2.tile([P, 128], F32, tag="ra")
                nc.scalar.activation(ra[:, 0:Gmax], po2[:, 0:Gmax], RELU,
                                     bias=rob2_t[:, 0:1], scale=1.0)
                ab = wk2.tile([P, 128], F32, tag="ab")
                nc.scalar.activation(ab[:, 0:Gmax], po2[:, 0:Gmax], ABS,
                                     bias=rob2_t[:, 0:1], scale=1.0)
                en = wk2.tile([P, 128], F32, tag="en")
                nc.scalar.activation(en[:, 0:Gmax], ab[:, 0:Gmax], EXP,
                                     bias=0.0, scale=-1.0)
                ul = wk2.tile([P, 128], F32, tag="ul")
                nc.scalar.activation(ul[:, 0:Gmax], en[:, 0:Gmax], LN,
                                     bias=1.0, scale=1.0)
                so2 = wk2.tile([P, 128], F32, tag="so2")
                nc.vector.tensor_tensor(so2[:, 0:Gmax], ra[:, 0:Gmax],
                                        ul[:, 0:Gmax], op=ADD)
                pout = ph2.tile([Gmax, 512], F32, tag="ro", name="roout")[:, 0:1]
                nc.tensor.matmul(pout[:], lhsT=so2[:, 0:Gmax], rhs=row3_t[:],
                                 start=True, stop=True)
                fout = wk2.tile([Gmax, 1], F32, tag="fout")
                nc.vector.tensor_scalar_add(fout[:], pout[:], b3_eff)
                nc.sync.dma_start(d_out[:], fout[:])

    nc.compile()
    return nc


def kernel(**inputs):
    z = np.asarray(inputs["z"]).astype(np.int64)
    edge_src = np.asarray(inputs["edge_src"]).astype(np.int64)
    edge_dst = np.asarray(inputs["edge_dst"]).astype(np.int64)
    batch = np.asarray(inputs["batch"]).astype(np.int64)
    G = int(inputs["num_graphs"])
    edge_weight = np.asarray(inputs["edge_weight"], np.float32)
    edge_attr = np.asarray(inputs["edge_attr"], np.float32)

    meta, pca = _host_prep(z, edge_src, edge_dst, batch, G, edge_weight, edge_attr)

    mlp_w1 = np.asarray(inputs["mlp_w1"], np.float32)
    mlp_b1 = np.asarray(inputs["mlp_b1"], np.float32)
    mlp_w2 = np.asarray(inputs["mlp_w2"], np.float32)
    mlp_b2 = np.asarray(inputs["mlp_b2"], np.float32)
    cf_w1 = np.asarray(inputs["cf_w1"], np.float32)
    cf_w2 = np.asarray(inputs["cf_w2"], np.float32)
    cf_b2 = np.asarray(inputs["cf_b2"], np.float32)
    lin_w = np.asarray(inputs["lin_w"], np.float32)
    lin_b = np.asarray(inputs["lin_b"], np.float32)
    ro_w1 = np.asarray(inputs["ro_w1"], np.float32)
    ro_b1 = np.asarray(inputs["ro_b1"], np.float32)
    ro_w2 = np.asarray(inputs["ro_w2"], np.float32)
    ro_b2 = np.asarray(inputs["ro_b2"], np.float32)
    ro_w3 = np.asarray(inputs["ro_w3"], np.float32)
    ro_b3 = np.asarray(inputs["ro_b3"], np.float32)

    b2_eff = mlp_b2 - LOG2 * mlp_w2.sum(axis=1)          # [L, H]
    cfw2b = cf_w2 * b2_eff[:, :, None]                   # scaled copy
    linb_eff = lin_b - LOG2 * lin_w.sum(axis=1)          # [L, H]
    b3_eff = float(ro_b3[0] - LOG2 * ro_w3.sum())

    weights = dict(b3_eff=b3_eff)
    nc = _build(meta, weights)

    shared = {
        "emb": np.asarray(inputs["emb"], np.float32),
        "w1": mlp_w1.astype(BF16),
        "b1": mlp_b1.reshape(L, H, 1),
        "w2": mlp_w2.astype(BF16),
        "cfw1": cf_w1,
        "cfw2": cf_w2,
        "cfw2b": cfw2b,
        "cfb2": cf_b2.reshape(L, H, 1),
        "linw": lin_w,
        "linb": linb_eff.reshape(L, H, 1),
        "row1": ro_w1,
        "rob1": np.tile(ro_b1[None, :], (P, 1)),
        "row2": ro_w2.reshape(5, H, H),
        "rob2": ro_b2.reshape(H, 1),
        "row3": ro_w3,
        "ident": np.eye(P, dtype=np.float32),
    }
    in_maps = []
    for c in range(NC):
        m = dict(shared)
        m["ea_g"] = pca[c]["ea_g"]
        m["ind_g"] = pca[c]["ind_g"]
        m["idx_w"] = pca[c]["idx_w"]
        m["zq"] = pca[c]["zq"]
        m["gind"] = pca[c]["gind"]
        in_maps.append(m)

    res = bass_utils.run_bass_kernel_spmd(nc, in_maps, core_ids=list(range(NC)))

    g_bound = meta["g_bound"]
    out = np.zeros((G, 1), np.float32)
    for c in range(NC):
        gs, ge = g_bound[c], g_bound[c + 1]
        out[gs:ge] = res.results[c]["out"][: ge - gs]
    return out



# revision 11
# speedup vs baseline: 1.5860x; 1.5860x over previous
"""SchNet-style GNN message passing on 8 Trainium2 NeuronCores.

Strategy (pure data parallel over the graph batch, per sharding hint):
- Nodes are split into 8 contiguous, graph-aligned ranges (batch is sorted).
- Each edge is owned by the core owning its dst node; per-core edges are
  sorted by dst and tiled into 128-message tiles that each fit a 128-node
  "window" of the destination range.
- Per layer: every core computes x = h_own @ cf_w1 for its own nodes,
  AllGathers the bf16 x-table (row layout) across cores, bulk-gathers
  x[src] rows with dma_gather (int16 indices => the table is addressed in
  a lo half [<32768] and a hi half; edges are processed in two phases),
  runs the filter MLP on-chip, multiplies, and scatter-adds messages via
  one-hot indicator matmuls on the PE (indicators are host-built, with the
  cosine cutoff C folded in).
- The filter bias b2 (with the softplus -log2 shift folded in) is added to
  the filter output inside PSUM via a K=1 matmul that pre-fills the psum
  accumulator before the ss@w2 matmuls accumulate on top.
- All activations (exp/ln softplus pairs, relu, abs) are pinned to the
  natural_log_exp_and_others activation table set so the scalar engine
  never reloads tables between Exp and Ln.
- Gathers move 1024 rows per op and round-robin across 4 SWDGE queues so
  descriptor generation and DMA flight overlap.
- Readout (segment-sum over graphs + MLP) runs locally per core.
"""

import numpy as np
import ml_dtypes

import bass_rust as _bass_rust
import concourse.bacc as bacc
import concourse.bass as bass
import concourse.tile as tile
from concourse import mybir
from concourse import bass_utils
from concourse.hw_specs import get_activation_tables
from concourse.library_config import mlp as _mlp_lib

BF16 = ml_dtypes.bfloat16
P = 128
H = 128
NGAUSS = 50
L = 3
CUTOFF = 10.0
LOG2 = float(np.log(2.0))
NC = 8
SPLIT = 32768
GROUP = 4          # message tiles per compute group (512 edges)
GG = 8             # tiles per dma_gather op (1024 rows <= desc ring)
GB = 16            # tiles per ea/ind DMA batch
F32 = mybir.dt.float32
BF = mybir.dt.bfloat16
I16 = mybir.dt.int16
I32 = mybir.dt.int32


class Bacc1T(bacc.Bacc):
    """Bacc that pins all activations to the natural_log_exp_and_others
    table set so alternating Exp/Ln activations never reload act tables."""

    def insert_act_table_loads(self):
        has_activation = any(
            isinstance(i, mybir.InstActivation)
            for b in self.main_func.blocks
            for i in b.instructions
        )
        if not has_activation:
            return
        keep = "natural_log_exp_and_others"
        tables = [
            (n, (s if n == keep else set()))
            for n, s in get_activation_tables(self.m.arch).items()
        ]
        _bass_rust.insert_act_table_loads(self, tables)


def _b2row_hilo(b2_eff):
    """[L, 2, GROUP*H] bf16: row0 = bf16(b2), row1 = bf16(b2 - f32(row0))."""
    tiled = np.tile(b2_eff[:, None, :], (1, 1, GROUP)).reshape(L, GROUP * H)
    hi = tiled.astype(BF16)
    lo = (tiled - hi.astype(np.float32)).astype(BF16)
    return np.stack([hi, lo], axis=1)


def _wrap16(vals, ncols):
    """dma_gather index layout: [16, n/16] wrapped, replicated to 128 partitions."""
    a = np.zeros((16, ncols), np.int16)
    n = len(vals)
    a[np.arange(n) % 16, np.arange(n) // 16] = vals.astype(np.int16)
    return np.tile(a, (8, 1))


def _host_prep(z, edge_src, edge_dst, batch, G, edge_weight, edge_attr):
    N = z.shape[0]
    E = edge_src.shape[0]

    counts = np.bincount(batch, minlength=G)
    cum = np.concatenate([[0], np.cumsum(counts)])  # node start of each graph
    # graph-aligned node boundaries, balanced by node count
    g_bound = np.zeros(NC + 1, np.int64)
    g_bound[NC] = G
    for c in range(1, NC):
        g_bound[c] = np.searchsorted(cum, c * N / NC)
    n_bound = cum[g_bound]

    n_own = np.diff(n_bound)
    NP = int(np.ceil(n_own.max() / P) * P)          # padded nodes per core
    W_CNT = NP // P
    Gmax = int(np.diff(g_bound).max())

    owner = np.searchsorted(n_bound, np.arange(N), side="right") - 1
    local = np.arange(N) - n_bound[owner]
    table_row = owner * NP + local                   # row in allgathered x table

    C_all = (0.5 * (np.cos(edge_weight * np.pi / CUTOFF) + 1.0)).astype(np.float32)

    e_owner = owner[edge_dst]
    src_row = table_row[edge_src]
    lo_mask = src_row < SPLIT

    # per (core, phase, window) edge counts -> uniform tiles per window
    T_pw = [0, 0]
    per_core = []
    for c in range(NC):
        sel = np.nonzero(e_owner == c)[0]
        ldst = local[edge_dst[sel]]
        order = np.argsort(ldst, kind="stable")
        sel = sel[order]
        ldst = ldst[order]
        win = ldst // P
        lo = lo_mask[sel]
        per_core.append((sel, ldst, win, lo))
        for ph in range(2):
            m = lo if ph == 0 else ~lo
            cnt = np.bincount(win[m], minlength=W_CNT)
            T_pw[ph] = max(T_pw[ph], int(np.ceil(cnt.max() / P)))
    T_pw = [max(t, 1) for t in T_pw]
    # pad each phase's tile count to a multiple of GB (ea/ind batch size)
    NT_A = int(np.ceil(W_CNT * T_pw[0] / GB) * GB)
    NT_B = int(np.ceil(W_CNT * T_pw[1] / GB) * GB)
    N_T = NT_A + NT_B

    meta = dict(NP=NP, W_CNT=W_CNT, Gmax=Gmax, T_pw=T_pw, NT_A=NT_A, NT_B=NT_B,
                N_T=N_T, n_bound=n_bound, g_bound=g_bound)

    per_core_arrays = []
    for c in range(NC):
        sel, ldst, win, lo = per_core[c]
        ea_full = np.zeros((N_T * P, NGAUSS), np.float32)
        ind_full = np.zeros((N_T * P, P), np.float32)
        src_full = np.zeros(N_T * P, np.int64)

        for ph in range(2):
            m = lo if ph == 0 else ~lo
            e_idx = sel[m]
            w_ph = win[m]
            l_ph = ldst[m]
            tpw = T_pw[ph]
            base = 0 if ph == 0 else NT_A
            # position within window (edges already window-sorted)
            cnt = np.bincount(w_ph, minlength=W_CNT)
            startw = np.concatenate([[0], np.cumsum(cnt)])
            k = np.arange(len(e_idx)) - startw[w_ph]
            slot = (base + w_ph * tpw + k // P) * P + (k % P)
            ea_full[slot] = edge_attr[e_idx]
            ind_full[slot, l_ph - w_ph * P] = C_all[e_idx]
            sr = src_row[e_idx]
            src_full[slot] = np.where(m[m], sr - (0 if ph == 0 else SPLIT), 0)

        # batch-major layouts
        # ea_b: [NGB, 50, GB*128] bf16 (transposed per batch)
        ea_b = (ea_full.reshape(N_T // GB, GB * P, NGAUSS)
                .transpose(0, 2, 1).astype(BF16))
        # ind_b: [NGB, 128, GB*128] bf16 : [p, tloc*128+col] = ind[tile, p, col]
        ind_b = (ind_full.reshape(N_T // GB, GB, P, P)
                 .transpose(0, 2, 1, 3).reshape(N_T // GB, P, GB * P)
                 .astype(BF16))
        # gather idx wrapped per GG tiles: [128, NGGRP*GG*128/16]
        npg = GG * P // 16
        idx_w = np.zeros((P, (N_T // GG) * npg), np.int16)
        for g in range(N_T // GG):
            idx_w[:, g * npg:(g + 1) * npg] = _wrap16(
                src_full[g * GG * P:(g + 1) * GG * P], npg)

        # node init: z indices [128, W_CNT]
        ns, ne = n_bound[c], n_bound[c + 1]
        zq = np.zeros(NP, np.int64)
        zq[: ne - ns] = z[ns:ne]
        zq = zq.reshape(W_CNT, P).T.astype(np.int32).copy()

        # graph indicator [W_CNT, 128, Gmax]
        gs, ge = g_bound[c], g_bound[c + 1]
        gi = np.zeros((NP, Gmax), np.float32)
        gl = batch[ns:ne] - gs
        gi[np.arange(ne - ns), gl] = 1.0
        gind = gi.reshape(W_CNT, P, Gmax)

        per_core_arrays.append(dict(ea_b=ea_b, ind_b=ind_b, idx_w=idx_w,
                                    zq=zq, gind=gind))
    return meta, per_core_arrays


def _build(meta, weights):
    NP, W_CNT, Gmax = meta["NP"], meta["W_CNT"], meta["Gmax"]
    NT_A, NT_B, N_T = meta["NT_A"], meta["NT_B"], meta["N_T"]
    npg = GG * P // 16

    nc = Bacc1T("TRN2", target_bir_lowering=False, debug=False,
                enable_asserts=False, num_devices=NC, num_swdge_queues=4)

    d_ea = nc.dram_tensor("ea_b", [N_T // GB, NGAUSS, GB * P], BF, kind="ExternalInput")
    d_ind = nc.dram_tensor("ind_b", [N_T // GB, P, GB * P], BF, kind="ExternalInput")
    d_idx = nc.dram_tensor("idx_w", [P, (N_T // GG) * npg], I16, kind="ExternalInput")
    d_zq = nc.dram_tensor("zq", [P, W_CNT], I32, kind="ExternalInput")
    d_gind = nc.dram_tensor("gind", [W_CNT, P, Gmax], F32, kind="ExternalInput")
    d_emb = nc.dram_tensor("emb", [120, H], F32, kind="ExternalInput")
    d_w1 = nc.dram_tensor("w1", [L, NGAUSS, H], BF, kind="ExternalInput")
    d_b1 = nc.dram_tensor("b1", [L, H, 1], F32, kind="ExternalInput")
    d_w2 = nc.dram_tensor("w2", [L, H, H], BF, kind="ExternalInput")
    d_b2row = nc.dram_tensor("b2row", [L, 2, GROUP * H], BF, kind="ExternalInput")
    d_ones = nc.dram_tensor("ones_row", [2, P], BF, kind="ExternalInput")
    d_cfw1 = nc.dram_tensor("cfw1", [L, H, H], F32, kind="ExternalInput")
    d_cfw2 = nc.dram_tensor("cfw2", [L, H, H], F32, kind="ExternalInput")
    d_cfb2 = nc.dram_tensor("cfb2", [L, H, 1], F32, kind="ExternalInput")
    d_linw = nc.dram_tensor("linw", [L, H, H], F32, kind="ExternalInput")
    d_linb = nc.dram_tensor("linb", [L, H, 1], F32, kind="ExternalInput")
    d_row1 = nc.dram_tensor("row1", [H, 5 * H], F32, kind="ExternalInput")
    d_rob1 = nc.dram_tensor("rob1", [P, 5 * H], F32, kind="ExternalInput")
    d_row2 = nc.dram_tensor("row2", [5, H, H], F32, kind="ExternalInput")
    d_rob2 = nc.dram_tensor("rob2", [H, 1], F32, kind="ExternalInput")
    d_row3 = nc.dram_tensor("row3", [H, 1], F32, kind="ExternalInput")
    d_ident = nc.dram_tensor("ident", [P, P], F32, kind="ExternalInput")
    d_out = nc.dram_tensor("out", [Gmax, 1], F32, kind="ExternalOutput")
    d_hdump = nc.dram_tensor("hdump", [P, NP], F32, kind="ExternalOutput")
    b3_eff = weights["b3_eff"]

    EXP = mybir.ActivationFunctionType.Exp
    LN = mybir.ActivationFunctionType.Ln
    RELU = mybir.ActivationFunctionType.Relu
    ABS = mybir.ActivationFunctionType.Abs
    MUL = mybir.AluOpType.mult
    ADD = mybir.AluOpType.add

    with tile.TileContext(nc) as tc:
        with tc.tile_pool(name="const", bufs=1) as cst, \
             tc.tile_pool(name="big", bufs=1) as big, \
             tc.tile_pool(name="dram", bufs=1, space="DRAM") as drp:

            nc.gpsimd.load_library(_mlp_lib)

            # resident tiles
            idx_t = cst.tile([P, (N_T // GG) * npg], I16)
            nc.sync.dma_start(idx_t[:], d_idx[:])
            zq_t = cst.tile([P, W_CNT], I32)
            nc.sync.dma_start(zq_t[:], d_zq[:])
            ident_t = cst.tile([P, P], F32)
            nc.sync.dma_start(ident_t[:], d_ident[:])
            ones_t = cst.tile([2, P], BF)
            nc.sync.dma_start(ones_t[:], d_ones[:])
            w1_t = [cst.tile([NGAUSS, H], BF, tag=f"w1_{l}", name=f"w1_{l}") for l in range(L)]
            b1_t = [cst.tile([H, 1], F32, tag=f"b1_{l}", name=f"b1_{l}") for l in range(L)]
            w2_t = [cst.tile([H, H], BF, tag=f"w2_{l}", name=f"w2_{l}") for l in range(L)]
            b2r_t = [cst.tile([2, GROUP * H], BF, tag=f"b2r_{l}", name=f"b2r_{l}") for l in range(L)]
            cfw1_t = [cst.tile([H, H], F32, tag=f"cfw1_{l}", name=f"cfw1_{l}") for l in range(L)]
            cfw2_t = [cst.tile([H, H], F32, tag=f"cfw2_{l}", name=f"cfw2_{l}") for l in range(L)]
            cfb2_t = [cst.tile([H, 1], F32, tag=f"cfb2_{l}", name=f"cfb2_{l}") for l in range(L)]
            linw_t = [cst.tile([H, H], F32, tag=f"linw_{l}", name=f"linw_{l}") for l in range(L)]
            linb_t = [cst.tile([H, 1], F32, tag=f"linb_{l}", name=f"linb_{l}") for l in range(L)]
            for l in range(L):
                nc.sync.dma_start(w1_t[l][:], d_w1[l])
                nc.sync.dma_start(b1_t[l][:], d_b1[l])
                nc.sync.dma_start(w2_t[l][:], d_w2[l])
                nc.sync.dma_start(b2r_t[l][:], d_b2row[l])
                nc.sync.dma_start(cfw1_t[l][:], d_cfw1[l])
                nc.sync.dma_start(cfw2_t[l][:], d_cfw2[l])
                nc.sync.dma_start(cfb2_t[l][:], d_cfb2[l])
                nc.sync.dma_start(linw_t[l][:], d_linw[l])
                nc.sync.dma_start(linb_t[l][:], d_linb[l])
            row1_t = cst.tile([H, 5 * H], F32)
            nc.sync.dma_start(row1_t[:], d_row1[:])
            rob1_t = cst.tile([P, 5 * H], F32)
            nc.sync.dma_start(rob1_t[:], d_rob1[:])
            row2_t = [cst.tile([H, H], F32, tag=f"row2_{i}", name=f"row2_{i}") for i in range(5)]
            for i in range(5):
                nc.sync.dma_start(row2_t[i][:], d_row2[i])
            rob2_t = cst.tile([H, 1], F32)
            nc.sync.dma_start(rob2_t[:], d_rob2[:])
            row3_t = cst.tile([H, 1], F32)
            nc.sync.dma_start(row3_t[:], d_row3[:])

            hT = big.tile([P, NP], F32)            # h_own^T
            aggA = big.tile([P, W_CNT * P], F32)   # aggT per window, phase A
            aggB = big.tile([P, W_CNT * P], F32)
            x_st = big.tile([P, W_CNT * H], BF)    # x_own rows staging

            x_own_ds = [drp.tile([NP, H], BF, name=f"x_own_{l}", tag=f"x_own_{l}")
                        for l in range(L)]
            x_full_ds = [drp.tile([NC * NP, H], BF, addr_space="Shared",
                                  name=f"x_full_{l}", tag=f"x_full_{l}")
                         for l in range(L)]

            with tc.tile_pool(name="pp", bufs=4, space="PSUM") as pp, \
                 tc.tile_pool(name="psca", bufs=2, space="PSUM") as psca, \
                 tc.tile_pool(name="wk", bufs=3) as wk:

                # ---- h0 = relu(emb[z])^T ----
                for j in range(W_CNT):
                    rows = wk.tile([P, H], F32, tag="h0rows")
                    nc.gpsimd.indirect_dma_start(
                        out=rows[:], out_offset=None, in_=d_emb[:],
                        in_offset=bass.IndirectOffsetOnAxis(ap=zq_t[:, j:j + 1], axis=0))
                    nc.vector.tensor_scalar_max(rows[:], rows[:], 0.0)
                    pt = pp.tile([P, 512], F32, tag="pp", name="pxt")[:, 0:128]
                    nc.tensor.transpose(pt[:], rows[:], ident_t[:])
                    nc.vector.tensor_copy(hT[:, j * P:(j + 1) * P], pt[:])

                gctr = 0
                for l in range(L):
                    x_own_d = x_own_ds[l]
                    x_full_d = x_full_ds[l]
                    # ---- x_own = h_own @ cf_w1[l]  (f32 matmul, rows, -> bf16) ----
                    for j0 in range(0, W_CNT, 4):
                        jn = min(4, W_CNT - j0)
                        pxt = pp.tile([P, 512], F32, tag="pp", name="pxt2")
                        for jj in range(jn):
                            nc.tensor.matmul(
                                pxt[:, jj * H:(jj + 1) * H],
                                lhsT=hT[:, (j0 + jj) * P:(j0 + jj + 1) * P],
                                rhs=cfw1_t[l][:], start=True, stop=True)
                        nc.vector.tensor_copy(
                            x_st[:, j0 * H:(j0 + jn) * H], pxt[:, :jn * H])
                    nc.sync.dma_start(
                        x_own_d[:].rearrange("(w p) h -> p w h", p=P),
                        x_st[:].rearrange("p (w h) -> p w h", h=H))
                    nc.gpsimd.collective_compute(
                        "AllGather", mybir.AluOpType.bypass,
                        replica_groups=[list(range(NC))],
                        ins=[x_own_d.opt()], outs=[x_full_d.opt()])

                    # ---- edge phases ----
                    for ph in range(2):
                        NT = NT_A if ph == 0 else NT_B
                        t_off = 0 if ph == 0 else NT_A
                        tpw = meta["T_pw"][ph]
                        tbl = x_full_d[:SPLIT, :] if ph == 0 else x_full_d[SPLIT:, :]
                        agg = aggA if ph == 0 else aggB
                        cur_a = None
                        ea_b = None
                        xg8 = None
                        for g in range(NT // GROUP):
                            if g % (GB // GROUP) == 0:
                                gb = (t_off + g * GROUP) // GB
                                ea_b = wk.tile([NGAUSS, GB * P], BF, tag="ea")
                                nc.sync.dma_start(ea_b[:], d_ea[gb])
                                ind_b = wk.tile([P, GB * P], BF, tag="ind")
                                nc.sync.dma_start(ind_b[:], d_ind[gb])
                            if g % (GG // GROUP) == 0:
                                g8 = (t_off + g * GROUP) // (GG * P) * P  # unused
                                gg = (t_off + g * GROUP) // (GG)
                                xg8 = wk.tile([P, GG * H], BF, tag="xg", bufs=4)
                                nc.gpsimd.dma_gather(
                                    xg8[:].rearrange("p (k h) -> p k h", h=H),
                                    tbl, idx_t[:, gg * npg:(gg + 1) * npg],
                                    GG * P, GG * P, H, queue_num=gctr % 4)
                                gctr += 1
                            go = (g % (GB // GROUP)) * GROUP * P   # col offset in ea_b/ind_b
                            xo = (g % (GG // GROUP)) * GROUP * H   # col offset in xg8
                            p1 = pp.tile([P, GROUP * P], F32, tag="pp", name="p1t")
                            nc.tensor.matmul(p1[:], lhsT=w1_t[l][:],
                                             rhs=ea_b[:, go:go + GROUP * P],
                                             start=True, stop=True)
                            e1 = wk.tile([P, GROUP * P], F32, tag="e1")
                            nc.scalar.activation(e1[:], p1[:], EXP,
                                                 bias=b1_t[l][:, 0:1], scale=1.0)
                            ss = wk.tile([P, GROUP * P], BF, tag="ss")
                            nc.scalar.activation(ss[:], e1[:], LN,
                                                 bias=1.0, scale=1.0)
                            p2 = pp.tile([P, GROUP * P], F32, tag="pp", name="p2t")
                            nc.tensor.matmul(p2[:], lhsT=ones_t[:],
                                             rhs=b2r_t[l][:],
                                             start=True, stop=False)
                            for tt in range(GROUP):
                                nc.tensor.matmul(
                                    p2[:, tt * H:(tt + 1) * H],
                                    lhsT=ss[:, tt * P:(tt + 1) * P],
                                    rhs=w2_t[l][:], start=False, stop=True)
                            msg = wk.tile([P, GROUP * H], BF, tag="msg")
                            nc.vector.tensor_tensor(msg[:], p2[:],
                                                    xg8[:, xo:xo + GROUP * H], op=MUL)
                            for tt in range(GROUP):
                                tl = g * GROUP + tt
                                if tl >= W_CNT * tpw:
                                    break
                                w = tl // tpw
                                first = (tl % tpw == 0)
                                last = (tl % tpw == tpw - 1)
                                if first:
                                    cur_a = psca.tile([P, 128], F32, tag="sa", name="sa")
                                nc.tensor.matmul(
                                    cur_a[:],
                                    lhsT=msg[:, tt * H:(tt + 1) * H],
                                    rhs=ind_b[:, go + tt * P:go + (tt + 1) * P],
                                    start=first, stop=last)
                                if last:
                                    nc.vector.tensor_copy(
                                        agg[:, w * P:(w + 1) * P], cur_a[:])

                    # ---- x2/x3 chain + h update ----
                    for j0 in range(0, W_CNT, 4):
                        jn = min(4, W_CNT - j0)
                        sl = slice(j0 * P, (j0 + jn) * P)
                        p2x = pp.tile([P, 512], F32, tag="pp", name="pxt3")
                        nc.tensor.matmul(p2x[:, :jn * P], lhsT=cfw2_t[l][:],
                                         rhs=aggA[:, sl],
                                         start=True, stop=False)
                        nc.tensor.matmul(p2x[:, :jn * P], lhsT=cfw2_t[l][:],
                                         rhs=aggB[:, sl],
                                         start=False, stop=True)
                        e2 = wk.tile([P, 512], F32, tag="e2", bufs=2)
                        nc.scalar.activation(e2[:, :jn * P], p2x[:, :jn * P], EXP,
                                             bias=cfb2_t[l][:, 0:1], scale=1.0)
                        s2 = wk.tile([P, 512], F32, tag="s2", bufs=2)
                        nc.scalar.activation(s2[:, :jn * P], e2[:, :jn * P], LN,
                                             bias=1.0, scale=1.0)
                        p3x = pp.tile([P, 512], F32, tag="pp", name="pxt4")
                        nc.tensor.matmul(p3x[:, :jn * P], lhsT=linw_t[l][:],
                                         rhs=s2[:, :jn * P], start=True, stop=True)
                        tmpu = wk.tile([P, 512], F32, tag="hupd", bufs=2)
                        nc.vector.tensor_scalar(
                            tmpu[:, :jn * P], p3x[:, :jn * P],
                            linb_t[l][:, 0:1], None, op0=ADD)
                        nc.vector.tensor_tensor(hT[:, sl], hT[:, sl],
                                                tmpu[:, :jn * P], op=ADD)

            nc.sync.dma_start(d_hdump[:], hT[:])

            # ---- readout ----
            with tc.tile_pool(name="pro", bufs=1, space="PSUM") as pro, \
                 tc.tile_pool(name="ph2", bufs=2, space="PSUM") as ph2, \
                 tc.tile_pool(name="wk2", bufs=2) as wk2:
                pooledT = pro.tile([P, 5 * 512], F32)   # 5 chunks x [128, Gmax<=128]
                for j in range(W_CNT):
                    ph_a = ph2.tile([P, 512], F32, tag="ro", name="roha")
                    ph_b = ph2.tile([P, 512], F32, tag="ro", name="rohb")[:, 0:128]
                    nc.tensor.matmul(ph_a[:], lhsT=hT[:, j * P:(j + 1) * P],
                                     rhs=row1_t[:, 0:512], start=True, stop=True)
                    nc.tensor.matmul(ph_b[:], lhsT=hT[:, j * P:(j + 1) * P],
                                     rhs=row1_t[:, 512:640], start=True, stop=True)
                    hhf = wk2.tile([P, 5 * H], F32, tag="hhf")
                    nc.vector.tensor_tensor(hhf[:, 0:512], ph_a[:],
                                            rob1_t[:, 0:512], op=ADD)
                    nc.vector.tensor_tensor(hhf[:, 512:640], ph_b[:],
                                            rob1_t[:, 512:640], op=ADD)
                    eh = wk2.tile([P, 5 * H], F32, tag="eh")
                    nc.scalar.activation(eh[:], hhf[:], EXP, bias=0.0, scale=1.0)
                    hh0 = wk2.tile([P, 5 * H], F32, tag="hh0")
                    nc.scalar.activation(hh0[:], eh[:], LN, bias=1.0, scale=1.0)
                    hh = wk2.tile([P, 5 * H], F32, tag="hh")
                    nc.vector.tensor_scalar_add(hh[:], hh0[:], -LOG2)
                    gi = wk2.tile([P, Gmax], F32, tag="gi")
                    nc.sync.dma_start(gi[:], d_gind[j])
                    for c5 in range(5):
                        nc.tensor.matmul(
                            pooledT[:, c5 * 512:c5 * 512 + Gmax],
                            lhsT=hh[:, c5 * H:(c5 + 1) * H], rhs=gi[:],
                            start=(j == 0), stop=(j == W_CNT - 1))
                plf = wk2.tile([P, 5 * P], F32, tag="plf")
                nc.vector.tensor_copy(
                    plf[:].rearrange("p (c g) -> p c g", g=P),
                    pooledT[:].rearrange("p (c g) -> p c g", g=512)[:, :, 0:P])
                po2 = ph2.tile([P, 512], F32, tag="ro", name="roo2")[:, 0:128]
                for c5 in range(5):
                    nc.tensor.matmul(po2[:, 0:Gmax], lhsT=row2_t[c5][:],
                                     rhs=plf[:, c5 * P:c5 * P + Gmax],
                                     start=(c5 == 0), stop=(c5 == 4))
                ra = wk2.tile([P, 128], F32, tag="ra")
                nc.scalar.activation(ra[:, 0:Gmax], po2[:, 0:Gmax], RELU,
                                     bias=rob2_t[:, 0:1], scale=1.0)
                ab = wk2.tile([P, 128], F32, tag="ab")
                nc.scalar.activation(ab[:, 0:Gmax], po2[:, 0:Gmax], ABS,
                                     bias=rob2_t[:, 0:1], scale=1.0)
                en = wk2.tile([P, 128], F32, tag="en")
                nc.scalar.activation(en[:, 0:Gmax], ab[:, 0:Gmax], EXP,
                                     bias=0.0, scale=-1.0)
                ul = wk2.tile([P, 128], F32, tag="ul")
                nc.scalar.activation(ul[:, 0:Gmax], en[:, 0:Gmax], LN,
                                     bias=1.0, scale=1.0)
                so2 = wk2.tile([P, 128], F32, tag="so2")
                nc.vector.tensor_tensor(so2[:, 0:Gmax], ra[:, 0:Gmax],
                                        ul[:, 0:Gmax], op=ADD)
                pout = ph2.tile([Gmax, 512], F32, tag="ro", name="roout")[:, 0:1]
                nc.tensor.matmul(pout[:], lhsT=so2[:, 0:Gmax], rhs=row3_t[:],
                                 start=True, stop=True)
                fout = wk2.tile([Gmax, 1], F32, tag="fout")
                nc.vector.tensor_scalar_add(fout[:], pout[:], b3_eff)
                nc.sync.dma_start(d_out[:], fout[:])

    nc.compile()
    return nc


def kernel(**inputs):
    z = np.asarray(inputs["z"]).astype(np.int64)
    edge_src = np.asarray(inputs["edge_src"]).astype(np.int64)
    edge_dst = np.asarray(inputs["edge_dst"]).astype(np.int64)
    batch = np.asarray(inputs["batch"]).astype(np.int64)
    G = int(inputs["num_graphs"])
    edge_weight = np.asarray(inputs["edge_weight"], np.float32)
    edge_attr = np.asarray(inputs["edge_attr"], np.float32)

    meta, pca = _host_prep(z, edge_src, edge_dst, batch, G, edge_weight, edge_attr)

    mlp_w1 = np.asarray(inputs["mlp_w1"], np.float32)
    mlp_b1 = np.asarray(inputs["mlp_b1"], np.float32)
    mlp_w2 = np.asarray(inputs["mlp_w2"], np.float32)
    mlp_b2 = np.asarray(inputs["mlp_b2"], np.float32)
    cf_w1 = np.asarray(inputs["cf_w1"], np.float32)
    cf_w2 = np.asarray(inputs["cf_w2"], np.float32)
    cf_b2 = np.asarray(inputs["cf_b2"], np.float32)
    lin_w = np.asarray(inputs["lin_w"], np.float32)
    lin_b = np.asarray(inputs["lin_b"], np.float32)
    ro_w1 = np.asarray(inputs["ro_w1"], np.float32)
    ro_b1 = np.asarray(inputs["ro_b1"], np.float32)
    ro_w2 = np.asarray(inputs["ro_w2"], np.float32)
    ro_b2 = np.asarray(inputs["ro_b2"], np.float32)
    ro_w3 = np.asarray(inputs["ro_w3"], np.float32)
    ro_b3 = np.asarray(inputs["ro_b3"], np.float32)

    b2_eff = mlp_b2 - LOG2 * mlp_w2.sum(axis=1)          # [L, H]
    linb_eff = lin_b - LOG2 * lin_w.sum(axis=1)          # [L, H]
    b3_eff = float(ro_b3[0] - LOG2 * ro_w3.sum())

    weights = dict(b3_eff=b3_eff)
    nc = _build(meta, weights)

    shared = {
        "emb": np.asarray(inputs["emb"], np.float32),
        "w1": mlp_w1.astype(BF16),
        "b1": mlp_b1.reshape(L, H, 1),
        "w2": mlp_w2.astype(BF16),
        "b2row": _b2row_hilo(b2_eff),
        "ones_row": np.ones((2, P), BF16),
        "cfw1": cf_w1,
        "cfw2": cf_w2,
        "cfb2": cf_b2.reshape(L, H, 1),
        "linw": lin_w,
        "linb": linb_eff.reshape(L, H, 1),
        "row1": ro_w1,
        "rob1": np.tile(ro_b1[None, :], (P, 1)),
        "row2": ro_w2.reshape(5, H, H),
        "rob2": ro_b2.reshape(H, 1),
        "row3": ro_w3,
        "ident": np.eye(P, dtype=np.float32),
    }
    in_maps = []
    for c in range(NC):
        m = dict(shared)
        m["ea_b"] = pca[c]["ea_b"]
        m["ind_b"] = pca[c]["ind_b"]
        m["idx_w"] = pca[c]["idx_w"]
        m["zq"] = pca[c]["zq"]
        m["gind"] = pca[c]["gind"]
        in_maps.append(m)

    res = bass_utils.run_bass_kernel_spmd(nc, in_maps, core_ids=list(range(NC)))

    import os as _os
    if _os.environ.get("KDBG"):
        np.save("/tmp/hdump.npy", res.results[0]["hdump"])
        np.save("/tmp/nbound.npy", meta["n_bound"])

    g_bound = meta["g_bound"]
    out = np.zeros((G, 1), np.float32)
    for c in range(NC):
        gs, ge = g_bound[c], g_bound[c + 1]
        out[gs:ge] = res.results[c]["out"][: ge - gs]
    return out


# revision 12
# speedup vs baseline: 1.6129x; 1.0170x over previous
"""SchNet-style GNN message passing on 8 Trainium2 NeuronCores.

Strategy (pure data parallel over the graph batch, per sharding hint):
- Nodes are split into 8 contiguous, graph-aligned ranges (batch is sorted).
- Each edge is owned by the core owning its dst node; per-core edges are
  sorted by dst and tiled into 128-message tiles that each fit a 128-node
  "window" of the destination range.
- Per layer: every core computes x = h_own @ cf_w1 for its own nodes,
  AllGathers the bf16 x-table (row layout) across cores, bulk-gathers
  x[src] rows with dma_gather (int16 indices => the table is addressed in
  a lo half [<32768] and a hi half; edges are processed in two phases),
  runs the filter MLP on-chip, multiplies, and scatter-adds messages via
  one-hot indicator matmuls on the PE (indicators are host-built, with the
  cosine cutoff C folded in).
- The filter bias b2 (with the softplus -log2 shift folded in) is added to
  the filter output inside PSUM via a K=1 matmul that pre-fills the psum
  accumulator before the ss@w2 matmuls accumulate on top.
- All activations (exp/ln softplus pairs, relu, abs) are pinned to the
  natural_log_exp_and_others activation table set so the scalar engine
  never reloads tables between Exp and Ln.
- Gathers move 1024 rows per op and round-robin across 4 SWDGE queues so
  descriptor generation and DMA flight overlap.
- Readout (segment-sum over graphs + MLP) runs locally per core.
"""

import numpy as np
import ml_dtypes

import bass_rust as _bass_rust
import concourse.bacc as bacc
import concourse.bass as bass
import concourse.tile as tile
from concourse import mybir
from concourse import bass_utils
from concourse.hw_specs import get_activation_tables
from concourse.library_config import mlp as _mlp_lib

BF16 = ml_dtypes.bfloat16
P = 128
H = 128
NGAUSS = 50
L = 3
CUTOFF = 10.0
LOG2 = float(np.log(2.0))
NC = 8
SPLIT = 32768
GROUP = 4          # message tiles per compute group (512 edges)
GG = 8             # tiles per dma_gather op (1024 rows <= desc ring)
GB = 16            # tiles per ea/ind DMA batch
F32 = mybir.dt.float32
BF = mybir.dt.bfloat16
I16 = mybir.dt.int16
I32 = mybir.dt.int32


class Bacc1T(bacc.Bacc):
    """Bacc that pins all activations to the natural_log_exp_and_others
    table set so alternating Exp/Ln activations never reload act tables."""

    def insert_act_table_loads(self):
        has_activation = any(
            isinstance(i, mybir.InstActivation)
            for b in self.main_func.blocks
            for i in b.instructions
        )
        if not has_activation:
            return
        keep = "natural_log_exp_and_others"
        tables = [
            (n, (s if n == keep else set()))
            for n, s in get_activation_tables(self.m.arch).items()
        ]
        _bass_rust.insert_act_table_loads(self, tables)


def _b2row_bcast(b2_eff):
    """[L, P, GROUP*H] f32: b2 tiled along H, broadcast across partitions."""
    tiled = np.tile(b2_eff[:, None, :], (1, 1, GROUP)).reshape(L, 1, GROUP * H)
    return np.tile(tiled, (1, P, 1)).astype(np.float32)


def _wrap16(vals, ncols):
    """dma_gather index layout: [16, n/16] wrapped, replicated to 128 partitions."""
    a = np.zeros((16, ncols), np.int16)
    n = len(vals)
    a[np.arange(n) % 16, np.arange(n) // 16] = vals.astype(np.int16)
    return np.tile(a, (8, 1))


def _host_prep(z, edge_src, edge_dst, batch, G, edge_weight, edge_attr):
    N = z.shape[0]
    E = edge_src.shape[0]

    counts = np.bincount(batch, minlength=G)
    cum = np.concatenate([[0], np.cumsum(counts)])  # node start of each graph
    # graph-aligned node boundaries, balanced by node count
    g_bound = np.zeros(NC + 1, np.int64)
    g_bound[NC] = G
    for c in range(1, NC):
        g_bound[c] = np.searchsorted(cum, c * N / NC)
    n_bound = cum[g_bound]

    n_own = np.diff(n_bound)
    NP = int(np.ceil(n_own.max() / P) * P)          # padded nodes per core
    W_CNT = NP // P
    Gmax = int(np.diff(g_bound).max())

    owner = np.searchsorted(n_bound, np.arange(N), side="right") - 1
    local = np.arange(N) - n_bound[owner]
    table_row = owner * NP + local                   # row in allgathered x table

    C_all = (0.5 * (np.cos(edge_weight * np.pi / CUTOFF) + 1.0)).astype(np.float32)

    e_owner = owner[edge_dst]
    src_row = table_row[edge_src]
    lo_mask = src_row < SPLIT

    # per (core, phase, window) edge counts -> uniform tiles per window
    T_pw = [0, 0]
    per_core = []
    for c in range(NC):
        sel = np.nonzero(e_owner == c)[0]
        ldst = local[edge_dst[sel]]
        order = np.argsort(ldst, kind="stable")
        sel = sel[order]
        ldst = ldst[order]
        win = ldst // P
        lo = lo_mask[sel]
        per_core.append((sel, ldst, win, lo))
        for ph in range(2):
            m = lo if ph == 0 else ~lo
            cnt = np.bincount(win[m], minlength=W_CNT)
            T_pw[ph] = max(T_pw[ph], int(np.ceil(cnt.max() / P)))
    T_pw = [max(t, 1) for t in T_pw]
    # pad each phase's tile count to a multiple of GB (ea/ind batch size)
    NT_A = int(np.ceil(W_CNT * T_pw[0] / GB) * GB)
    NT_B = int(np.ceil(W_CNT * T_pw[1] / GB) * GB)
    N_T = NT_A + NT_B

    meta = dict(NP=NP, W_CNT=W_CNT, Gmax=Gmax, T_pw=T_pw, NT_A=NT_A, NT_B=NT_B,
                N_T=N_T, n_bound=n_bound, g_bound=g_bound)

    per_core_arrays = []
    for c in range(NC):
        sel, ldst, win, lo = per_core[c]
        ea_full = np.zeros((N_T * P, NGAUSS), np.float32)
        ind_full = np.zeros((N_T * P, P), np.float32)
        src_full = np.zeros(N_T * P, np.int64)

        for ph in range(2):
            m = lo if ph == 0 else ~lo
            e_idx = sel[m]
            w_ph = win[m]
            l_ph = ldst[m]
            tpw = T_pw[ph]
            base = 0 if ph == 0 else NT_A
            # position within window (edges already window-sorted)
            cnt = np.bincount(w_ph, minlength=W_CNT)
            startw = np.concatenate([[0], np.cumsum(cnt)])
            k = np.arange(len(e_idx)) - startw[w_ph]
            slot = (base + w_ph * tpw + k // P) * P + (k % P)
            ea_full[slot] = edge_attr[e_idx]
            ind_full[slot, l_ph - w_ph * P] = C_all[e_idx]
            sr = src_row[e_idx]
            src_full[slot] = np.where(m[m], sr - (0 if ph == 0 else SPLIT), 0)

        # batch-major layouts
        # ea_b: [NGB, 50, GB*128] bf16 (transposed per batch)
        ea_b = (ea_full.reshape(N_T // GB, GB * P, NGAUSS)
                .transpose(0, 2, 1).astype(BF16))
        # ind_b: [NGB, 128, GB*128] bf16 : [p, tloc*128+col] = ind[tile, p, col]
        ind_b = (ind_full.reshape(N_T // GB, GB, P, P)
                 .transpose(0, 2, 1, 3).reshape(N_T // GB, P, GB * P)
                 .astype(BF16))
        # gather idx wrapped per GG tiles: [128, NGGRP*GG*128/16]
        npg = GG * P // 16
        idx_w = np.zeros((P, (N_T // GG) * npg), np.int16)
        for g in range(N_T // GG):
            idx_w[:, g * npg:(g + 1) * npg] = _wrap16(
                src_full[g * GG * P:(g + 1) * GG * P], npg)

        # node init: z indices [128, W_CNT]
        ns, ne = n_bound[c], n_bound[c + 1]
        zq = np.zeros(NP, np.int64)
        zq[: ne - ns] = z[ns:ne]
        zq = zq.reshape(W_CNT, P).T.astype(np.int32).copy()

        # graph indicator [W_CNT, 128, Gmax]
        gs, ge = g_bound[c], g_bound[c + 1]
        gi = np.zeros((NP, Gmax), np.float32)
        gl = batch[ns:ne] - gs
        gi[np.arange(ne - ns), gl] = 1.0
        gind = gi.reshape(W_CNT, P, Gmax)

        per_core_arrays.append(dict(ea_b=ea_b, ind_b=ind_b, idx_w=idx_w,
                                    zq=zq, gind=gind))
    return meta, per_core_arrays


def _build(meta, weights):
    NP, W_CNT, Gmax = meta["NP"], meta["W_CNT"], meta["Gmax"]
    NT_A, NT_B, N_T = meta["NT_A"], meta["NT_B"], meta["N_T"]
    npg = GG * P // 16

    nc = Bacc1T("TRN2", target_bir_lowering=False, debug=False,
                enable_asserts=False, num_devices=NC, num_swdge_queues=4)

    d_ea = nc.dram_tensor("ea_b", [N_T // GB, NGAUSS, GB * P], BF, kind="ExternalInput")
    d_ind = nc.dram_tensor("ind_b", [N_T // GB, P, GB * P], BF, kind="ExternalInput")
    d_idx = nc.dram_tensor("idx_w", [P, (N_T // GG) * npg], I16, kind="ExternalInput")
    d_zq = nc.dram_tensor("zq", [P, W_CNT], I32, kind="ExternalInput")
    d_gind = nc.dram_tensor("gind", [W_CNT, P, Gmax], F32, kind="ExternalInput")
    d_emb = nc.dram_tensor("emb", [120, H], F32, kind="ExternalInput")
    d_w1 = nc.dram_tensor("w1", [L, NGAUSS, H], BF, kind="ExternalInput")
    d_b1 = nc.dram_tensor("b1", [L, H, 1], F32, kind="ExternalInput")
    d_w2 = nc.dram_tensor("w2", [L, H, H], BF, kind="ExternalInput")
    d_b2row = nc.dram_tensor("b2row", [L, P, GROUP * H], F32, kind="ExternalInput")
    d_cfw1 = nc.dram_tensor("cfw1", [L, H, H], F32, kind="ExternalInput")
    d_cfw2 = nc.dram_tensor("cfw2", [L, H, H], F32, kind="ExternalInput")
    d_cfb2 = nc.dram_tensor("cfb2", [L, H, 1], F32, kind="ExternalInput")
    d_linw = nc.dram_tensor("linw", [L, H, H], F32, kind="ExternalInput")
    d_linb = nc.dram_tensor("linb", [L, H, 1], F32, kind="ExternalInput")
    d_row1 = nc.dram_tensor("row1", [H, 5 * H], F32, kind="ExternalInput")
    d_rob1 = nc.dram_tensor("rob1", [P, 5 * H], F32, kind="ExternalInput")
    d_row2 = nc.dram_tensor("row2", [5, H, H], F32, kind="ExternalInput")
    d_rob2 = nc.dram_tensor("rob2", [H, 1], F32, kind="ExternalInput")
    d_row3 = nc.dram_tensor("row3", [H, 1], F32, kind="ExternalInput")
    d_ident = nc.dram_tensor("ident", [P, P], F32, kind="ExternalInput")
    d_out = nc.dram_tensor("out", [Gmax, 1], F32, kind="ExternalOutput")
    d_hdump = nc.dram_tensor("hdump", [P, NP], F32, kind="ExternalOutput")
    b3_eff = weights["b3_eff"]

    EXP = mybir.ActivationFunctionType.Exp
    LN = mybir.ActivationFunctionType.Ln
    RELU = mybir.ActivationFunctionType.Relu
    ABS = mybir.ActivationFunctionType.Abs
    MUL = mybir.AluOpType.mult
    ADD = mybir.AluOpType.add

    with tile.TileContext(nc) as tc:
        with tc.tile_pool(name="const", bufs=1) as cst, \
             tc.tile_pool(name="big", bufs=1) as big, \
             tc.tile_pool(name="dram", bufs=1, space="DRAM") as drp:

            nc.gpsimd.load_library(_mlp_lib)

            # resident tiles
            idx_t = cst.tile([P, (N_T // GG) * npg], I16)
            nc.sync.dma_start(idx_t[:], d_idx[:])
            zq_t = cst.tile([P, W_CNT], I32)
            nc.sync.dma_start(zq_t[:], d_zq[:])
            ident_t = cst.tile([P, P], F32)
            nc.sync.dma_start(ident_t[:], d_ident[:])
            w1_t = [cst.tile([NGAUSS, H], BF, tag=f"w1_{l}", name=f"w1_{l}") for l in range(L)]
            b1_t = [cst.tile([H, 1], F32, tag=f"b1_{l}", name=f"b1_{l}") for l in range(L)]
            w2_t = [cst.tile([H, H], BF, tag=f"w2_{l}", name=f"w2_{l}") for l in range(L)]
            b2r_t = [cst.tile([P, GROUP * H], F32, tag=f"b2r_{l}", name=f"b2r_{l}") for l in range(L)]
            cfw1_t = [cst.tile([H, H], F32, tag=f"cfw1_{l}", name=f"cfw1_{l}") for l in range(L)]
            cfw2_t = [cst.tile([H, H], F32, tag=f"cfw2_{l}", name=f"cfw2_{l}") for l in range(L)]
            cfb2_t = [cst.tile([H, 1], F32, tag=f"cfb2_{l}", name=f"cfb2_{l}") for l in range(L)]
            linw_t = [cst.tile([H, H], F32, tag=f"linw_{l}", name=f"linw_{l}") for l in range(L)]
            linb_t = [cst.tile([H, 1], F32, tag=f"linb_{l}", name=f"linb_{l}") for l in range(L)]
            for l in range(L):
                nc.sync.dma_start(w1_t[l][:], d_w1[l])
                nc.sync.dma_start(b1_t[l][:], d_b1[l])
                nc.sync.dma_start(w2_t[l][:], d_w2[l])
                nc.sync.dma_start(b2r_t[l][:], d_b2row[l])
                nc.sync.dma_start(cfw1_t[l][:], d_cfw1[l])
                nc.sync.dma_start(cfw2_t[l][:], d_cfw2[l])
                nc.sync.dma_start(cfb2_t[l][:], d_cfb2[l])
                nc.sync.dma_start(linw_t[l][:], d_linw[l])
                nc.sync.dma_start(linb_t[l][:], d_linb[l])
            row1_t = cst.tile([H, 5 * H], F32)
            nc.sync.dma_start(row1_t[:], d_row1[:])
            rob1_t = cst.tile([P, 5 * H], F32)
            nc.sync.dma_start(rob1_t[:], d_rob1[:])
            row2_t = [cst.tile([H, H], F32, tag=f"row2_{i}", name=f"row2_{i}") for i in range(5)]
            for i in range(5):
                nc.sync.dma_start(row2_t[i][:], d_row2[i])
            rob2_t = cst.tile([H, 1], F32)
            nc.sync.dma_start(rob2_t[:], d_rob2[:])
            row3_t = cst.tile([H, 1], F32)
            nc.sync.dma_start(row3_t[:], d_row3[:])

            hT = big.tile([P, NP], F32)            # h_own^T
            aggA = big.tile([P, W_CNT * P], F32)   # aggT per window, phase A
            aggB = big.tile([P, W_CNT * P], F32)
            x_st = big.tile([P, W_CNT * H], BF)    # x_own rows staging

            x_own_ds = [drp.tile([NP, H], BF, name=f"x_own_{l}", tag=f"x_own_{l}")
                        for l in range(L)]
            x_full_ds = [drp.tile([NC * NP, H], BF, addr_space="Shared",
                                  name=f"x_full_{l}", tag=f"x_full_{l}")
                         for l in range(L)]

            with tc.tile_pool(name="pp", bufs=4, space="PSUM") as pp, \
                 tc.tile_pool(name="psca", bufs=2, space="PSUM") as psca, \
                 tc.tile_pool(name="wk", bufs=3) as wk:

                # ---- h0 = relu(emb[z])^T ----
                for j in range(W_CNT):
                    rows = wk.tile([P, H], F32, tag="h0rows")
                    nc.gpsimd.indirect_dma_start(
                        out=rows[:], out_offset=None, in_=d_emb[:],
                        in_offset=bass.IndirectOffsetOnAxis(ap=zq_t[:, j:j + 1], axis=0))
                    nc.vector.tensor_scalar_max(rows[:], rows[:], 0.0)
                    pt = pp.tile([P, 512], F32, tag="pp", name="pxt")[:, 0:128]
                    nc.tensor.transpose(pt[:], rows[:], ident_t[:])
                    nc.vector.tensor_copy(hT[:, j * P:(j + 1) * P], pt[:])

                gctr = 0
                for l in range(L):
                    x_own_d = x_own_ds[l]
                    x_full_d = x_full_ds[l]
                    # ---- x_own = h_own @ cf_w1[l]  (f32 matmul, rows, -> bf16) ----
                    for j0 in range(0, W_CNT, 4):
                        jn = min(4, W_CNT - j0)
                        pxt = pp.tile([P, 512], F32, tag="pp", name="pxt2")
                        for jj in range(jn):
                            nc.tensor.matmul(
                                pxt[:, jj * H:(jj + 1) * H],
                                lhsT=hT[:, (j0 + jj) * P:(j0 + jj + 1) * P],
                                rhs=cfw1_t[l][:], start=True, stop=True)
                        nc.vector.tensor_copy(
                            x_st[:, j0 * H:(j0 + jn) * H], pxt[:, :jn * H])
                    nc.sync.dma_start(
                        x_own_d[:].rearrange("(w p) h -> p w h", p=P),
                        x_st[:].rearrange("p (w h) -> p w h", h=H))
                    nc.gpsimd.collective_compute(
                        "AllGather", mybir.AluOpType.bypass,
                        replica_groups=[list(range(NC))],
                        ins=[x_own_d.opt()], outs=[x_full_d.opt()])

                    # ---- edge phases ----
                    for ph in range(2):
                        NT = NT_A if ph == 0 else NT_B
                        t_off = 0 if ph == 0 else NT_A
                        tpw = meta["T_pw"][ph]
                        tbl = x_full_d[:SPLIT, :] if ph == 0 else x_full_d[SPLIT:, :]
                        agg = aggA if ph == 0 else aggB
                        cur_a = None
                        ea_b = None
                        xg8 = None
                        for g in range(NT // GROUP):
                            if g % (GB // GROUP) == 0:
                                gb = (t_off + g * GROUP) // GB
                                ea_b = wk.tile([NGAUSS, GB * P], BF, tag="ea")
                                nc.sync.dma_start(ea_b[:], d_ea[gb])
                                ind_b = wk.tile([P, GB * P], BF, tag="ind")
                                nc.sync.dma_start(ind_b[:], d_ind[gb])
                            if g % (GG // GROUP) == 0:
                                g8 = (t_off + g * GROUP) // (GG * P) * P  # unused
                                gg = (t_off + g * GROUP) // (GG)
                                xg8 = wk.tile([P, GG * H], BF, tag="xg", bufs=4)
                                nc.gpsimd.dma_gather(
                                    xg8[:].rearrange("p (k h) -> p k h", h=H),
                                    tbl, idx_t[:, gg * npg:(gg + 1) * npg],
                                    GG * P, GG * P, H, queue_num=gctr % 4)
                                gctr += 1
                            go = (g % (GB // GROUP)) * GROUP * P   # col offset in ea_b/ind_b
                            xo = (g % (GG // GROUP)) * GROUP * H   # col offset in xg8
                            p1 = pp.tile([P, GROUP * P], F32, tag="pp", name="p1t")
                            nc.tensor.matmul(p1[:], lhsT=w1_t[l][:],
                                             rhs=ea_b[:, go:go + GROUP * P],
                                             start=True, stop=True)
                            e1 = wk.tile([P, GROUP * P], F32, tag="e1")
                            nc.scalar.activation(e1[:], p1[:], EXP,
                                                 bias=b1_t[l][:, 0:1], scale=1.0)
                            ss = wk.tile([P, GROUP * P], BF, tag="ss")
                            nc.scalar.activation(ss[:], e1[:], LN,
                                                 bias=1.0, scale=1.0)
                            p2 = pp.tile([P, GROUP * P], F32, tag="pp", name="p2t")
                            for tt in range(GROUP):
                                nc.tensor.matmul(
                                    p2[:, tt * H:(tt + 1) * H],
                                    lhsT=ss[:, tt * P:(tt + 1) * P],
                                    rhs=w2_t[l][:], start=True, stop=True)
                            p2b = wk.tile([P, GROUP * P], F32, tag="p2b")
                            nc.vector.tensor_tensor(p2b[:], p2[:],
                                                    b2r_t[l][:], op=ADD)
                            msg = wk.tile([P, GROUP * H], BF, tag="msg")
                            nc.vector.tensor_tensor(msg[:], p2b[:],
                                                    xg8[:, xo:xo + GROUP * H], op=MUL)
                            for tt in range(GROUP):
                                tl = g * GROUP + tt
                                if tl >= W_CNT * tpw:
                                    break
                                w = tl // tpw
                                first = (tl % tpw == 0)
                                last = (tl % tpw == tpw - 1)
                                if first:
                                    cur_a = psca.tile([P, 128], F32, tag="sa", name="sa")
                                nc.tensor.matmul(
                                    cur_a[:],
                                    lhsT=msg[:, tt * H:(tt + 1) * H],
                                    rhs=ind_b[:, go + tt * P:go + (tt + 1) * P],
                                    start=first, stop=last)
                                if last:
                                    nc.vector.tensor_copy(
                                        agg[:, w * P:(w + 1) * P], cur_a[:])

                    # ---- x2/x3 chain + h update ----
                    for j0 in range(0, W_CNT, 4):
                        jn = min(4, W_CNT - j0)
                        sl = slice(j0 * P, (j0 + jn) * P)
                        p2x = pp.tile([P, 512], F32, tag="pp", name="pxt3")
                        nc.tensor.matmul(p2x[:, :jn * P], lhsT=cfw2_t[l][:],
                                         rhs=aggA[:, sl],
                                         start=True, stop=False)
                        nc.tensor.matmul(p2x[:, :jn * P], lhsT=cfw2_t[l][:],
                                         rhs=aggB[:, sl],
                                         start=False, stop=True)
                        e2 = wk.tile([P, 512], F32, tag="e2", bufs=2)
                        nc.scalar.activation(e2[:, :jn * P], p2x[:, :jn * P], EXP,
                                             bias=cfb2_t[l][:, 0:1], scale=1.0)
                        s2 = wk.tile([P, 512], F32, tag="s2", bufs=2)
                        nc.scalar.activation(s2[:, :jn * P], e2[:, :jn * P], LN,
                                             bias=1.0, scale=1.0)
                        p3x = pp.tile([P, 512], F32, tag="pp", name="pxt4")
                        nc.tensor.matmul(p3x[:, :jn * P], lhsT=linw_t[l][:],
                                         rhs=s2[:, :jn * P], start=True, stop=True)
                        tmpu = wk.tile([P, 512], F32, tag="hupd", bufs=2)
                        nc.vector.tensor_scalar(
                            tmpu[:, :jn * P], p3x[:, :jn * P],
                            linb_t[l][:, 0:1], None, op0=ADD)
                        nc.vector.tensor_tensor(hT[:, sl], hT[:, sl],
                                                tmpu[:, :jn * P], op=ADD)

            nc.sync.dma_start(d_hdump[:], hT[:])

            # ---- readout ----
            with tc.tile_pool(name="pro", bufs=1, space="PSUM") as pro, \
                 tc.tile_pool(name="ph2", bufs=2, space="PSUM") as ph2, \
                 tc.tile_pool(name="wk2", bufs=2) as wk2:
                pooledT = pro.tile([P, 5 * 512], F32)   # 5 chunks x [128, Gmax<=128]
                for j in range(W_CNT):
                    ph_a = ph2.tile([P, 512], F32, tag="ro", name="roha")
                    ph_b = ph2.tile([P, 512], F32, tag="ro", name="rohb")[:, 0:128]
                    nc.tensor.matmul(ph_a[:], lhsT=hT[:, j * P:(j + 1) * P],
                                     rhs=row1_t[:, 0:512], start=True, stop=True)
                    nc.tensor.matmul(ph_b[:], lhsT=hT[:, j * P:(j + 1) * P],
                                     rhs=row1_t[:, 512:640], start=True, stop=True)
                    hhf = wk2.tile([P, 5 * H], F32, tag="hhf")
                    nc.vector.tensor_tensor(hhf[:, 0:512], ph_a[:],
                                            rob1_t[:, 0:512], op=ADD)
                    nc.vector.tensor_tensor(hhf[:, 512:640], ph_b[:],
                                            rob1_t[:, 512:640], op=ADD)
                    eh = wk2.tile([P, 5 * H], F32, tag="eh")
                    nc.scalar.activation(eh[:], hhf[:], EXP, bias=0.0, scale=1.0)
                    hh0 = wk2.tile([P, 5 * H], F32, tag="hh0")
                    nc.scalar.activation(hh0[:], eh[:], LN, bias=1.0, scale=1.0)
                    hh = wk2.tile([P, 5 * H], F32, tag="hh")
                    nc.vector.tensor_scalar_add(hh[:], hh0[:], -LOG2)
                    gi = wk2.tile([P, Gmax], F32, tag="gi")
                    nc.sync.dma_start(gi[:], d_gind[j])
                    for c5 in range(5):
                        nc.tensor.matmul(
                            pooledT[:, c5 * 512:c5 * 512 + Gmax],
                            lhsT=hh[:, c5 * H:(c5 + 1) * H], rhs=gi[:],
                            start=(j == 0), stop=(j == W_CNT - 1))
                plf = wk2.tile([P, 5 * P], F32, tag="plf")
                nc.vector.tensor_copy(
                    plf[:].rearrange("p (c g) -> p c g", g=P),
                    pooledT[:].rearrange("p (c g) -> p c g", g=512)[:, :, 0:P])
                po2 = ph2.tile([P, 512], F32, tag="ro", name="roo2")[:, 0:128]
                for c5 in range(5):
                    nc.tensor.matmul(po2[:, 0:Gmax], lhsT=row2_t[c5][:],
                                     rhs=plf[:, c5 * P:c5 * P + Gmax],
                                     start=(c5 == 0), stop=(c5 == 4))
                ra = wk2.tile([P, 128], F32, tag="ra")
                nc.scalar.activation(ra[:, 0:Gmax], po2[:, 0:Gmax], RELU,
                                     bias=rob2_t[:, 0:1], scale=1.0)
                ab = wk2.tile([P, 128], F32, tag="ab")
                nc.scalar.activation(ab[:, 0:Gmax], po2[:, 0:Gmax], ABS,
                                     bias=rob2_t[:, 0:1], scale=1.0)
                en = wk2.tile([P, 128], F32, tag="en")
                nc.scalar.activation(en[:, 0:Gmax], ab[:, 0:Gmax], EXP,
                                     bias=0.0, scale=-1.0)
                ul = wk2.tile([P, 128], F32, tag="ul")
                nc.scalar.activation(ul[:, 0:Gmax], en[:, 0:Gmax], LN,
                                     bias=1.0, scale=1.0)
                so2 = wk2.tile([P, 128], F32, tag="so2")
                nc.vector.tensor_tensor(so2[:, 0:Gmax], ra[:, 0:Gmax],
                                        ul[:, 0:Gmax], op=ADD)
                pout = ph2.tile([Gmax, 512], F32, tag="ro", name="roout")[:, 0:1]
                nc.tensor.matmul(pout[:], lhsT=so2[:, 0:Gmax], rhs=row3_t[:],
                                 start=True, stop=True)
                fout = wk2.tile([Gmax, 1], F32, tag="fout")
                nc.vector.tensor_scalar_add(fout[:], pout[:], b3_eff)
                nc.sync.dma_start(d_out[:], fout[:])

    nc.compile()
    return nc


def kernel(**inputs):
    z = np.asarray(inputs["z"]).astype(np.int64)
    edge_src = np.asarray(inputs["edge_src"]).astype(np.int64)
    edge_dst = np.asarray(inputs["edge_dst"]).astype(np.int64)
    batch = np.asarray(inputs["batch"]).astype(np.int64)
    G = int(inputs["num_graphs"])
    edge_weight = np.asarray(inputs["edge_weight"], np.float32)
    edge_attr = np.asarray(inputs["edge_attr"], np.float32)

    meta, pca = _host_prep(z, edge_src, edge_dst, batch, G, edge_weight, edge_attr)

    mlp_w1 = np.asarray(inputs["mlp_w1"], np.float32)
    mlp_b1 = np.asarray(inputs["mlp_b1"], np.float32)
    mlp_w2 = np.asarray(inputs["mlp_w2"], np.float32)
    mlp_b2 = np.asarray(inputs["mlp_b2"], np.float32)
    cf_w1 = np.asarray(inputs["cf_w1"], np.float32)
    cf_w2 = np.asarray(inputs["cf_w2"], np.float32)
    cf_b2 = np.asarray(inputs["cf_b2"], np.float32)
    lin_w = np.asarray(inputs["lin_w"], np.float32)
    lin_b = np.asarray(inputs["lin_b"], np.float32)
    ro_w1 = np.asarray(inputs["ro_w1"], np.float32)
    ro_b1 = np.asarray(inputs["ro_b1"], np.float32)
    ro_w2 = np.asarray(inputs["ro_w2"], np.float32)
    ro_b2 = np.asarray(inputs["ro_b2"], np.float32)
    ro_w3 = np.asarray(inputs["ro_w3"], np.float32)
    ro_b3 = np.asarray(inputs["ro_b3"], np.float32)

    b2_eff = mlp_b2 - LOG2 * mlp_w2.sum(axis=1)          # [L, H]
    linb_eff = lin_b - LOG2 * lin_w.sum(axis=1)          # [L, H]
    b3_eff = float(ro_b3[0] - LOG2 * ro_w3.sum())

    weights = dict(b3_eff=b3_eff)
    nc = _build(meta, weights)

    shared = {
        "emb": np.asarray(inputs["emb"], np.float32),
        "w1": mlp_w1.astype(BF16),
        "b1": mlp_b1.reshape(L, H, 1),
        "w2": mlp_w2.astype(BF16),
        "b2row": _b2row_bcast(b2_eff),
        "cfw1": cf_w1,
        "cfw2": cf_w2,
        "cfb2": cf_b2.reshape(L, H, 1),
        "linw": lin_w,
        "linb": linb_eff.reshape(L, H, 1),
        "row1": ro_w1,
        "rob1": np.tile(ro_b1[None, :], (P, 1)),
        "row2": ro_w2.reshape(5, H, H),
        "rob2": ro_b2.reshape(H, 1),
        "row3": ro_w3,
        "ident": np.eye(P, dtype=np.float32),
    }
    in_maps = []
    for c in range(NC):
        m = dict(shared)
        m["ea_b"] = pca[c]["ea_b"]
        m["ind_b"] = pca[c]["ind_b"]
        m["idx_w"] = pca[c]["idx_w"]
        m["zq"] = pca[c]["zq"]
        m["gind"] = pca[c]["gind"]
        in_maps.append(m)

    res = bass_utils.run_bass_kernel_spmd(nc, in_maps, core_ids=list(range(NC)))

    import os as _os
    if _os.environ.get("KDBG"):
        np.save("/tmp/hdump.npy", res.results[0]["hdump"])
        np.save("/tmp/nbound.npy", meta["n_bound"])

    g_bound = meta["g_bound"]
    out = np.zeros((G, 1), np.float32)
    for c in range(NC):
        gs, ge = g_bound[c], g_bound[c + 1]
        out[gs:ge] = res.results[c]["out"][: ge - gs]
    return out


# revision 13
# speedup vs baseline: 1.8195x; 1.1281x over previous
"""SchNet-style GNN message passing on 8 Trainium2 NeuronCores.

Strategy (pure data parallel over the graph batch, per sharding hint):
- Nodes are split into 8 contiguous, graph-aligned ranges (batch is sorted).
- Each edge is owned by the core owning its dst node; per-core edges are
  sorted by dst and tiled into 128-message tiles that each fit a 128-node
  "window" of the destination range.
- Per layer: every core computes x = h_own @ cf_w1 for its own nodes,
  AllGathers the bf16 x-table (row layout) across cores, bulk-gathers
  x[src] rows with dma_gather (int16 indices => the table is addressed in
  a lo half [<32768] and a hi half; edges are processed in two phases),
  runs the filter MLP on-chip, multiplies, and scatter-adds messages via
  one-hot indicator matmuls on the PE (indicators are host-built, with the
  cosine cutoff C folded in).
- The filter bias b2 (with the softplus -log2 shift folded in) is added to
  the filter output inside PSUM via a K=1 matmul that pre-fills the psum
  accumulator before the ss@w2 matmuls accumulate on top.
- All activations (exp/ln softplus pairs, relu, abs) are pinned to the
  natural_log_exp_and_others activation table set so the scalar engine
  never reloads tables between Exp and Ln.
- Gathers move 1024 rows per op and round-robin across 4 SWDGE queues so
  descriptor generation and DMA flight overlap.
- Readout (segment-sum over graphs + MLP) runs locally per core.
"""

import numpy as np
import ml_dtypes

import bass_rust as _bass_rust
import concourse.bacc as bacc
import concourse.bass as bass
import concourse.tile as tile
from concourse import mybir
from concourse import bass_utils
from concourse.hw_specs import get_activation_tables
from concourse.library_config import mlp as _mlp_lib

BF16 = ml_dtypes.bfloat16
P = 128
H = 128
NGAUSS = 50
L = 3
CUTOFF = 10.0
LOG2 = float(np.log(2.0))
NC = 8
SPLIT = 32768
GROUP = 4          # message tiles per compute group (512 edges)
GG = 8             # tiles per dma_gather op (1024 rows <= desc ring)
GB = 16            # tiles per ea/ind DMA batch
F32 = mybir.dt.float32
BF = mybir.dt.bfloat16
I16 = mybir.dt.int16
I32 = mybir.dt.int32


class Bacc1T(bacc.Bacc):
    """Bacc that pins all activations to the natural_log_exp_and_others
    table set so alternating Exp/Ln activations never reload act tables."""

    def insert_act_table_loads(self):
        has_activation = any(
            isinstance(i, mybir.InstActivation)
            for b in self.main_func.blocks
            for i in b.instructions
        )
        if not has_activation:
            return
        keep = "natural_log_exp_and_others"
        tables = [
            (n, (s if n == keep else set()))
            for n, s in get_activation_tables(self.m.arch).items()
        ]
        _bass_rust.insert_act_table_loads(self, tables)


def _b2row_bcast(b2_eff):
    """[L, P, GROUP*H] f32: b2 tiled along H, broadcast across partitions."""
    tiled = np.tile(b2_eff[:, None, :], (1, 1, GROUP)).reshape(L, 1, GROUP * H)
    return np.tile(tiled, (1, P, 1)).astype(np.float32)


def _wrap16(vals, ncols):
    """dma_gather index layout: [16, n/16] wrapped, replicated to 128 partitions."""
    a = np.zeros((16, ncols), np.int16)
    n = len(vals)
    a[np.arange(n) % 16, np.arange(n) // 16] = vals.astype(np.int16)
    return np.tile(a, (8, 1))


def _host_prep(z, edge_src, edge_dst, batch, G, edge_weight, edge_attr):
    N = z.shape[0]
    E = edge_src.shape[0]

    counts = np.bincount(batch, minlength=G)
    cum = np.concatenate([[0], np.cumsum(counts)])  # node start of each graph
    # graph-aligned node boundaries, balanced by node count
    g_bound = np.zeros(NC + 1, np.int64)
    g_bound[NC] = G
    for c in range(1, NC):
        g_bound[c] = np.searchsorted(cum, c * N / NC)
    n_bound = cum[g_bound]

    n_own = np.diff(n_bound)
    NP = int(np.ceil(n_own.max() / P) * P)          # padded nodes per core
    W_CNT = NP // P
    Gmax = int(np.diff(g_bound).max())

    owner = np.searchsorted(n_bound, np.arange(N), side="right") - 1
    local = np.arange(N) - n_bound[owner]
    table_row = owner * NP + local                   # row in allgathered x table

    C_all = (0.5 * (np.cos(edge_weight * np.pi / CUTOFF) + 1.0)).astype(np.float32)

    e_owner = owner[edge_dst]
    src_row = table_row[edge_src]
    lo_mask = src_row < SPLIT

    # per (core, phase, window) edge counts -> uniform tiles per window
    T_pw = [0, 0]
    per_core = []
    for c in range(NC):
        sel = np.nonzero(e_owner == c)[0]
        ldst = local[edge_dst[sel]]
        order = np.argsort(ldst, kind="stable")
        sel = sel[order]
        ldst = ldst[order]
        win = ldst // P
        lo = lo_mask[sel]
        per_core.append((sel, ldst, win, lo))
        for ph in range(2):
            m = lo if ph == 0 else ~lo
            cnt = np.bincount(win[m], minlength=W_CNT)
            T_pw[ph] = max(T_pw[ph], int(np.ceil(cnt.max() / P)))
    T_pw = [max(t, 1) for t in T_pw]
    # pad each phase's tile count to a multiple of GB (ea/ind batch size)
    NT_A = int(np.ceil(W_CNT * T_pw[0] / GB) * GB)
    NT_B = int(np.ceil(W_CNT * T_pw[1] / GB) * GB)
    N_T = NT_A + NT_B

    meta = dict(NP=NP, W_CNT=W_CNT, Gmax=Gmax, T_pw=T_pw, NT_A=NT_A, NT_B=NT_B,
                N_T=N_T, n_bound=n_bound, g_bound=g_bound)

    per_core_arrays = []
    for c in range(NC):
        sel, ldst, win, lo = per_core[c]
        ea_full = np.zeros((N_T * P, NGAUSS), np.float32)
        ind_full = np.zeros((N_T * P, P), np.float32)
        src_full = np.zeros(N_T * P, np.int64)

        for ph in range(2):
            m = lo if ph == 0 else ~lo
            e_idx = sel[m]
            w_ph = win[m]
            l_ph = ldst[m]
            tpw = T_pw[ph]
            base = 0 if ph == 0 else NT_A
            # position within window (edges already window-sorted)
            cnt = np.bincount(w_ph, minlength=W_CNT)
            startw = np.concatenate([[0], np.cumsum(cnt)])
            k = np.arange(len(e_idx)) - startw[w_ph]
            slot = (base + w_ph * tpw + k // P) * P + (k % P)
            ea_full[slot] = edge_attr[e_idx]
            ind_full[slot, l_ph - w_ph * P] = C_all[e_idx]
            sr = src_row[e_idx]
            src_full[slot] = np.where(m[m], sr - (0 if ph == 0 else SPLIT), 0)

        # batch-major layouts
        # ea_b: [NGB, 50, GB*128] bf16 (transposed per batch)
        ea_b = (ea_full.reshape(N_T // GB, GB * P, NGAUSS)
                .transpose(0, 2, 1).astype(BF16))
        # ind_b: [NGB, 128, GB*128] bf16 : [p, tloc*128+col] = ind[tile, p, col]
        ind_b = (ind_full.reshape(N_T // GB, GB, P, P)
                 .transpose(0, 2, 1, 3).reshape(N_T // GB, P, GB * P)
                 .astype(BF16))
        # gather idx wrapped per GG tiles: [128, NGGRP*GG*128/16]
        npg = GG * P // 16
        idx_w = np.zeros((P, (N_T // GG) * npg), np.int16)
        for g in range(N_T // GG):
            idx_w[:, g * npg:(g + 1) * npg] = _wrap16(
                src_full[g * GG * P:(g + 1) * GG * P], npg)

        # node init: z indices [128, W_CNT]
        ns, ne = n_bound[c], n_bound[c + 1]
        zq = np.zeros(NP, np.int64)
        zq[: ne - ns] = z[ns:ne]
        zq = zq.reshape(W_CNT, P).T.astype(np.int32).copy()

        # graph indicator [W_CNT, 128, Gmax]
        gs, ge = g_bound[c], g_bound[c + 1]
        gi = np.zeros((NP, Gmax), np.float32)
        gl = batch[ns:ne] - gs
        gi[np.arange(ne - ns), gl] = 1.0
        gind = gi.reshape(W_CNT, P, Gmax)

        per_core_arrays.append(dict(ea_b=ea_b, ind_b=ind_b, idx_w=idx_w,
                                    zq=zq, gind=gind))
    return meta, per_core_arrays


def _build(meta, weights):
    NP, W_CNT, Gmax = meta["NP"], meta["W_CNT"], meta["Gmax"]
    NT_A, NT_B, N_T = meta["NT_A"], meta["NT_B"], meta["N_T"]
    npg = GG * P // 16

    nc = Bacc1T("TRN2", target_bir_lowering=False, debug=False,
                enable_asserts=False, num_devices=NC, num_swdge_queues=4)

    d_ea = nc.dram_tensor("ea_b", [N_T // GB, NGAUSS, GB * P], BF, kind="ExternalInput")
    d_ind = nc.dram_tensor("ind_b", [N_T // GB, P, GB * P], BF, kind="ExternalInput")
    d_idx = nc.dram_tensor("idx_w", [P, (N_T // GG) * npg], I16, kind="ExternalInput")
    d_zq = nc.dram_tensor("zq", [P, W_CNT], I32, kind="ExternalInput")
    d_gind = nc.dram_tensor("gind", [W_CNT, P, Gmax], F32, kind="ExternalInput")
    d_emb = nc.dram_tensor("emb", [120, H], F32, kind="ExternalInput")
    d_w1 = nc.dram_tensor("w1", [L, NGAUSS, H], BF, kind="ExternalInput")
    d_b1 = nc.dram_tensor("b1", [L, H, 1], F32, kind="ExternalInput")
    d_w2 = nc.dram_tensor("w2", [L, H, H], BF, kind="ExternalInput")
    d_b2row = nc.dram_tensor("b2row", [L, P, GROUP * H], F32, kind="ExternalInput")
    d_cfw1 = nc.dram_tensor("cfw1", [L, H, H], F32, kind="ExternalInput")
    d_cfw2 = nc.dram_tensor("cfw2", [L, H, H], F32, kind="ExternalInput")
    d_cfb2 = nc.dram_tensor("cfb2", [L, H, 1], F32, kind="ExternalInput")
    d_linw = nc.dram_tensor("linw", [L, H, H], F32, kind="ExternalInput")
    d_linb = nc.dram_tensor("linb", [L, H, 1], F32, kind="ExternalInput")
    d_row1 = nc.dram_tensor("row1", [H, 5 * H], F32, kind="ExternalInput")
    d_rob1 = nc.dram_tensor("rob1", [P, 5 * H], F32, kind="ExternalInput")
    d_row2 = nc.dram_tensor("row2", [5, H, H], F32, kind="ExternalInput")
    d_rob2 = nc.dram_tensor("rob2", [H, 1], F32, kind="ExternalInput")
    d_row3 = nc.dram_tensor("row3", [H, 1], F32, kind="ExternalInput")
    d_ident = nc.dram_tensor("ident", [P, P], F32, kind="ExternalInput")
    d_out = nc.dram_tensor("out", [Gmax, 1], F32, kind="ExternalOutput")
    d_hdump = nc.dram_tensor("hdump", [P, NP], F32, kind="ExternalOutput")
    b3_eff = weights["b3_eff"]

    EXP = mybir.ActivationFunctionType.Exp
    LN = mybir.ActivationFunctionType.Ln
    RELU = mybir.ActivationFunctionType.Relu
    ABS = mybir.ActivationFunctionType.Abs
    MUL = mybir.AluOpType.mult
    ADD = mybir.AluOpType.add

    with tile.TileContext(nc) as tc:
        with tc.tile_pool(name="const", bufs=1) as cst, \
             tc.tile_pool(name="big", bufs=1) as big, \
             tc.tile_pool(name="dram", bufs=1, space="DRAM") as drp:

            nc.gpsimd.load_library(_mlp_lib)

            # resident tiles
            idx_t = cst.tile([P, (N_T // GG) * npg], I16)
            nc.sync.dma_start(idx_t[:], d_idx[:])
            zq_t = cst.tile([P, W_CNT], I32)
            nc.sync.dma_start(zq_t[:], d_zq[:])
            ident_t = cst.tile([P, P], F32)
            nc.sync.dma_start(ident_t[:], d_ident[:])
            w1_t = [cst.tile([NGAUSS, H], BF, tag=f"w1_{l}", name=f"w1_{l}") for l in range(L)]
            b1_t = [cst.tile([H, 1], F32, tag=f"b1_{l}", name=f"b1_{l}") for l in range(L)]
            w2_t = [cst.tile([H, H], BF, tag=f"w2_{l}", name=f"w2_{l}") for l in range(L)]
            b2r_t = [cst.tile([P, GROUP * H], F32, tag=f"b2r_{l}", name=f"b2r_{l}") for l in range(L)]
            cfw1_t = [cst.tile([H, H], F32, tag=f"cfw1_{l}", name=f"cfw1_{l}") for l in range(L)]
            cfw2_t = [cst.tile([H, H], F32, tag=f"cfw2_{l}", name=f"cfw2_{l}") for l in range(L)]
            cfb2_t = [cst.tile([H, 1], F32, tag=f"cfb2_{l}", name=f"cfb2_{l}") for l in range(L)]
            linw_t = [cst.tile([H, H], F32, tag=f"linw_{l}", name=f"linw_{l}") for l in range(L)]
            linb_t = [cst.tile([H, 1], F32, tag=f"linb_{l}", name=f"linb_{l}") for l in range(L)]
            for l in range(L):
                nc.sync.dma_start(w1_t[l][:], d_w1[l])
                nc.sync.dma_start(b1_t[l][:], d_b1[l])
                nc.sync.dma_start(w2_t[l][:], d_w2[l])
                nc.sync.dma_start(b2r_t[l][:], d_b2row[l])
                nc.sync.dma_start(cfw1_t[l][:], d_cfw1[l])
                nc.sync.dma_start(cfw2_t[l][:], d_cfw2[l])
                nc.sync.dma_start(cfb2_t[l][:], d_cfb2[l])
                nc.sync.dma_start(linw_t[l][:], d_linw[l])
                nc.sync.dma_start(linb_t[l][:], d_linb[l])
            row1_t = cst.tile([H, 5 * H], F32)
            nc.sync.dma_start(row1_t[:], d_row1[:])
            rob1_t = cst.tile([P, 5 * H], F32)
            nc.sync.dma_start(rob1_t[:], d_rob1[:])
            row2_t = [cst.tile([H, H], F32, tag=f"row2_{i}", name=f"row2_{i}") for i in range(5)]
            for i in range(5):
                nc.sync.dma_start(row2_t[i][:], d_row2[i])
            rob2_t = cst.tile([H, 1], F32)
            nc.sync.dma_start(rob2_t[:], d_rob2[:])
            row3_t = cst.tile([H, 1], F32)
            nc.sync.dma_start(row3_t[:], d_row3[:])

            hT = big.tile([P, NP], F32)            # h_own^T
            aggA = big.tile([P, W_CNT * P], F32)   # aggT per window, phase A
            aggB = big.tile([P, W_CNT * P], F32)
            x_st = big.tile([P, W_CNT * H], BF)    # x_own rows staging

            x_own_ds = [drp.tile([NP, H], BF, name=f"x_own_{l}", tag=f"x_own_{l}")
                        for l in range(L)]
            x_full_ds = [drp.tile([NC * NP, H], BF, addr_space="Shared",
                                  name=f"x_full_{l}", tag=f"x_full_{l}")
                         for l in range(L)]

            with tc.tile_pool(name="pp", bufs=4, space="PSUM") as pp, \
                 tc.tile_pool(name="psca", bufs=2, space="PSUM") as psca, \
                 tc.tile_pool(name="wk", bufs=3) as wk:

                # ---- h0 = relu(emb[z])^T ----
                for j in range(W_CNT):
                    rows = wk.tile([P, H], F32, tag="h0rows")
                    nc.gpsimd.indirect_dma_start(
                        out=rows[:], out_offset=None, in_=d_emb[:],
                        in_offset=bass.IndirectOffsetOnAxis(ap=zq_t[:, j:j + 1], axis=0))
                    nc.vector.tensor_scalar_max(rows[:], rows[:], 0.0)
                    pt = pp.tile([P, 512], F32, tag="pp", name="pxt")[:, 0:128]
                    nc.tensor.transpose(pt[:], rows[:], ident_t[:])
                    nc.vector.tensor_copy(hT[:, j * P:(j + 1) * P], pt[:])

                gctr = 0
                for l in range(L):
                    x_own_d = x_own_ds[l]
                    x_full_d = x_full_ds[l]
                    # ---- x_own = h_own @ cf_w1[l]  (f32 matmul, rows, -> bf16) ----
                    for j0 in range(0, W_CNT, 4):
                        jn = min(4, W_CNT - j0)
                        pxt = pp.tile([P, 512], F32, tag="pp", name="pxt2")
                        for jj in range(jn):
                            nc.tensor.matmul(
                                pxt[:, jj * H:(jj + 1) * H],
                                lhsT=hT[:, (j0 + jj) * P:(j0 + jj + 1) * P],
                                rhs=cfw1_t[l][:], start=True, stop=True)
                        nc.vector.tensor_copy(
                            x_st[:, j0 * H:(j0 + jn) * H], pxt[:, :jn * H])
                    nc.sync.dma_start(
                        x_own_d[:].rearrange("(w p) h -> p w h", p=P),
                        x_st[:].rearrange("p (w h) -> p w h", h=H))
                    nc.gpsimd.collective_compute(
                        "AllGather", mybir.AluOpType.bypass,
                        replica_groups=[list(range(NC))],
                        ins=[x_own_d.opt()], outs=[x_full_d.opt()])

                    # ---- edge phases (software-pipelined: p1(g) | p2(g-1) | scatter(g-2)) ----
                    for ph in range(2):
                        NT = NT_A if ph == 0 else NT_A + NT_B
                        g_lo = 0 if ph == 0 else NT_A // GROUP
                        tpw = meta["T_pw"][ph]
                        tbl = x_full_d[:SPLIT, :] if ph == 0 else x_full_d[SPLIT:, :]
                        agg = aggA if ph == 0 else aggB
                        NG_ph = NT // GROUP - g_lo
                        cur_a = None
                        stage = {}   # g -> dict of live tiles
                        for gi_ in range(NG_ph + 2):
                            g = g_lo + gi_
                            if gi_ < NG_ph:
                                if gi_ % (GB // GROUP) == 0:
                                    gb = g * GROUP // GB
                                    ea_b = wk.tile([NGAUSS, GB * P], BF, tag="ea")
                                    nc.sync.dma_start(ea_b[:], d_ea[gb])
                                    ind_b = wk.tile([P, GB * P], BF, tag="ind")
                                    nc.sync.dma_start(ind_b[:], d_ind[gb])
                                if gi_ % (GG // GROUP) == 0:
                                    gg = g * GROUP // GG
                                    xg8 = wk.tile([P, GG * H], BF, tag="xg", bufs=4)
                                    nc.gpsimd.dma_gather(
                                        xg8[:].rearrange("p (k h) -> p k h", h=H),
                                        tbl, idx_t[:, gg * npg:(gg + 1) * npg],
                                        GG * P, GG * P, H, queue_num=gctr % 4)
                                    gctr += 1
                                go = (gi_ % (GB // GROUP)) * GROUP * P
                                xo = (gi_ % (GG // GROUP)) * GROUP * H
                                p1 = pp.tile([P, GROUP * P], F32, tag="pp", name="p1t")
                                nc.tensor.matmul(p1[:], lhsT=w1_t[l][:],
                                                 rhs=ea_b[:, go:go + GROUP * P],
                                                 start=True, stop=True)
                                e1 = wk.tile([P, GROUP * P], F32, tag="e1")
                                nc.scalar.activation(e1[:], p1[:], EXP,
                                                     bias=b1_t[l][:, 0:1], scale=1.0)
                                ss = wk.tile([P, GROUP * P], BF, tag="ss")
                                nc.scalar.activation(ss[:], e1[:], LN,
                                                     bias=1.0, scale=1.0)
                                stage[gi_] = dict(ss=ss, xg8=xg8, xo=xo,
                                                  ind_b=ind_b, go=go)
                            if gi_ - 1 >= 0 and gi_ - 1 in stage:
                                st = stage[gi_ - 1]
                                p2 = pp.tile([P, GROUP * P], F32, tag="pp", name="p2t")
                                for tt in range(GROUP):
                                    nc.tensor.matmul(
                                        p2[:, tt * H:(tt + 1) * H],
                                        lhsT=st["ss"][:, tt * P:(tt + 1) * P],
                                        rhs=w2_t[l][:], start=True, stop=True)
                                p2b = wk.tile([P, GROUP * P], F32, tag="p2b")
                                nc.vector.tensor_tensor(p2b[:], p2[:],
                                                        b2r_t[l][:], op=ADD)
                                msg = wk.tile([P, GROUP * H], BF, tag="msg")
                                nc.vector.tensor_tensor(
                                    msg[:], p2b[:],
                                    st["xg8"][:, st["xo"]:st["xo"] + GROUP * H],
                                    op=MUL)
                                st["msg"] = msg
                            if gi_ - 2 >= 0 and gi_ - 2 in stage:
                                st = stage.pop(gi_ - 2)
                                msg = st["msg"]
                                ind_b2 = st["ind_b"]
                                go2 = st["go"]
                                for tt in range(GROUP):
                                    tl = (gi_ - 2) * GROUP + tt
                                    if tl >= W_CNT * tpw:
                                        break
                                    w = tl // tpw
                                    first = (tl % tpw == 0)
                                    last = (tl % tpw == tpw - 1)
                                    if first:
                                        cur_a = psca.tile([P, 128], F32, tag="sa", name="sa")
                                    nc.tensor.matmul(
                                        cur_a[:],
                                        lhsT=msg[:, tt * H:(tt + 1) * H],
                                        rhs=ind_b2[:, go2 + tt * P:go2 + (tt + 1) * P],
                                        start=first, stop=last)
                                    if last:
                                        nc.vector.tensor_copy(
                                            agg[:, w * P:(w + 1) * P], cur_a[:])

                    # ---- x2/x3 chain + h update ----
                    for j0 in range(0, W_CNT, 4):
                        jn = min(4, W_CNT - j0)
                        sl = slice(j0 * P, (j0 + jn) * P)
                        p2x = pp.tile([P, 512], F32, tag="pp", name="pxt3")
                        nc.tensor.matmul(p2x[:, :jn * P], lhsT=cfw2_t[l][:],
                                         rhs=aggA[:, sl],
                                         start=True, stop=False)
                        nc.tensor.matmul(p2x[:, :jn * P], lhsT=cfw2_t[l][:],
                                         rhs=aggB[:, sl],
                                         start=False, stop=True)
                        e2 = wk.tile([P, 512], F32, tag="e2", bufs=2)
                        nc.scalar.activation(e2[:, :jn * P], p2x[:, :jn * P], EXP,
                                             bias=cfb2_t[l][:, 0:1], scale=1.0)
                        s2 = wk.tile([P, 512], F32, tag="s2", bufs=2)
                        nc.scalar.activation(s2[:, :jn * P], e2[:, :jn * P], LN,
                                             bias=1.0, scale=1.0)
                        p3x = pp.tile([P, 512], F32, tag="pp", name="pxt4")
                        nc.tensor.matmul(p3x[:, :jn * P], lhsT=linw_t[l][:],
                                         rhs=s2[:, :jn * P], start=True, stop=True)
                        tmpu = wk.tile([P, 512], F32, tag="hupd", bufs=2)
                        nc.vector.tensor_scalar(
                            tmpu[:, :jn * P], p3x[:, :jn * P],
                            linb_t[l][:, 0:1], None, op0=ADD)
                        nc.vector.tensor_tensor(hT[:, sl], hT[:, sl],
                                                tmpu[:, :jn * P], op=ADD)

            nc.sync.dma_start(d_hdump[:], hT[:])

            # ---- readout ----
            with tc.tile_pool(name="pro", bufs=1, space="PSUM") as pro, \
                 tc.tile_pool(name="ph2", bufs=2, space="PSUM") as ph2, \
                 tc.tile_pool(name="wk2", bufs=2) as wk2:
                pooledT = pro.tile([P, 5 * 512], F32)   # 5 chunks x [128, Gmax<=128]
                for j in range(W_CNT):
                    ph_a = ph2.tile([P, 512], F32, tag="ro", name="roha")
                    ph_b = ph2.tile([P, 512], F32, tag="ro", name="rohb")[:, 0:128]
                    nc.tensor.matmul(ph_a[:], lhsT=hT[:, j * P:(j + 1) * P],
                                     rhs=row1_t[:, 0:512], start=True, stop=True)
                    nc.tensor.matmul(ph_b[:], lhsT=hT[:, j * P:(j + 1) * P],
                                     rhs=row1_t[:, 512:640], start=True, stop=True)
                    hhf = wk2.tile([P, 5 * H], F32, tag="hhf")
                    nc.vector.tensor_tensor(hhf[:, 0:512], ph_a[:],
                                            rob1_t[:, 0:512], op=ADD)
                    nc.vector.tensor_tensor(hhf[:, 512:640], ph_b[:],
                                            rob1_t[:, 512:640], op=ADD)
                    eh = wk2.tile([P, 5 * H], F32, tag="eh")
                    nc.scalar.activation(eh[:], hhf[:], EXP, bias=0.0, scale=1.0)
                    hh0 = wk2.tile([P, 5 * H], F32, tag="hh0")
                    nc.scalar.activation(hh0[:], eh[:], LN, bias=1.0, scale=1.0)
                    hh = wk2.tile([P, 5 * H], F32, tag="hh")
                    nc.vector.tensor_scalar_add(hh[:], hh0[:], -LOG2)
                    gi = wk2.tile([P, Gmax], F32, tag="gi")
                    nc.sync.dma_start(gi[:], d_gind[j])
                    for c5 in range(5):
                        nc.tensor.matmul(
                            pooledT[:, c5 * 512:c5 * 512 + Gmax],
                            lhsT=hh[:, c5 * H:(c5 + 1) * H], rhs=gi[:],
                            start=(j == 0), stop=(j == W_CNT - 1))
                plf = wk2.tile([P, 5 * P], F32, tag="plf")
                nc.vector.tensor_copy(
                    plf[:].rearrange("p (c g) -> p c g", g=P),
                    pooledT[:].rearrange("p (c g) -> p c g", g=512)[:, :, 0:P])
                po2 = ph2.tile([P, 512], F32, tag="ro", name="roo2")[:, 0:128]
                for c5 in range(5):
                    nc.tensor.matmul(po2[:, 0:Gmax], lhsT=row2_t[c5][:],
                                     rhs=plf[:, c5 * P:c5 * P + Gmax],
                                     start=(c5 == 0), stop=(c5 == 4))
                ra = wk2.tile([P, 128], F32, tag="ra")
                nc.scalar.activation(ra[:, 0:Gmax], po2[:, 0:Gmax], RELU,
                                     bias=rob2_t[:, 0:1], scale=1.0)
                ab = wk2.tile([P, 128], F32, tag="ab")
                nc.scalar.activation(ab[:, 0:Gmax], po2[:, 0:Gmax], ABS,
                                     bias=rob2_t[:, 0:1], scale=1.0)
                en = wk2.tile([P, 128], F32, tag="en")
                nc.scalar.activation(en[:, 0:Gmax], ab[:, 0:Gmax], EXP,
                                     bias=0.0, scale=-1.0)
                ul = wk2.tile([P, 128], F32, tag="ul")
                nc.scalar.activation(ul[:, 0:Gmax], en[:, 0:Gmax], LN,
                                     bias=1.0, scale=1.0)
                so2 = wk2.tile([P, 128], F32, tag="so2")
                nc.vector.tensor_tensor(so2[:, 0:Gmax], ra[:, 0:Gmax],
                                        ul[:, 0:Gmax], op=ADD)
                pout = ph2.tile([Gmax, 512], F32, tag="ro", name="roout")[:, 0:1]
                nc.tensor.matmul(pout[:], lhsT=so2[:, 0:Gmax], rhs=row3_t[:],
                                 start=True, stop=True)
                fout = wk2.tile([Gmax, 1], F32, tag="fout")
                nc.vector.tensor_scalar_add(fout[:], pout[:], b3_eff)
                nc.sync.dma_start(d_out[:], fout[:])

    nc.compile()
    return nc


def kernel(**inputs):
    z = np.asarray(inputs["z"]).astype(np.int64)
    edge_src = np.asarray(inputs["edge_src"]).astype(np.int64)
    edge_dst = np.asarray(inputs["edge_dst"]).astype(np.int64)
    batch = np.asarray(inputs["batch"]).astype(np.int64)
    G = int(inputs["num_graphs"])
    edge_weight = np.asarray(inputs["edge_weight"], np.float32)
    edge_attr = np.asarray(inputs["edge_attr"], np.float32)

    meta, pca = _host_prep(z, edge_src, edge_dst, batch, G, edge_weight, edge_attr)

    mlp_w1 = np.asarray(inputs["mlp_w1"], np.float32)
    mlp_b1 = np.asarray(inputs["mlp_b1"], np.float32)
    mlp_w2 = np.asarray(inputs["mlp_w2"], np.float32)
    mlp_b2 = np.asarray(inputs["mlp_b2"], np.float32)
    cf_w1 = np.asarray(inputs["cf_w1"], np.float32)
    cf_w2 = np.asarray(inputs["cf_w2"], np.float32)
    cf_b2 = np.asarray(inputs["cf_b2"], np.float32)
    lin_w = np.asarray(inputs["lin_w"], np.float32)
    lin_b = np.asarray(inputs["lin_b"], np.float32)
    ro_w1 = np.asarray(inputs["ro_w1"], np.float32)
    ro_b1 = np.asarray(inputs["ro_b1"], np.float32)
    ro_w2 = np.asarray(inputs["ro_w2"], np.float32)
    ro_b2 = np.asarray(inputs["ro_b2"], np.float32)
    ro_w3 = np.asarray(inputs["ro_w3"], np.float32)
    ro_b3 = np.asarray(inputs["ro_b3"], np.float32)

    b2_eff = mlp_b2 - LOG2 * mlp_w2.sum(axis=1)          # [L, H]
    linb_eff = lin_b - LOG2 * lin_w.sum(axis=1)          # [L, H]
    b3_eff = float(ro_b3[0] - LOG2 * ro_w3.sum())

    weights = dict(b3_eff=b3_eff)
    nc = _build(meta, weights)

    shared = {
        "emb": np.asarray(inputs["emb"], np.float32),
        "w1": mlp_w1.astype(BF16),
        "b1": mlp_b1.reshape(L, H, 1),
        "w2": mlp_w2.astype(BF16),
        "b2row": _b2row_bcast(b2_eff),
        "cfw1": cf_w1,
        "cfw2": cf_w2,
        "cfb2": cf_b2.reshape(L, H, 1),
        "linw": lin_w,
        "linb": linb_eff.reshape(L, H, 1),
        "row1": ro_w1,
        "rob1": np.tile(ro_b1[None, :], (P, 1)),
        "row2": ro_w2.reshape(5, H, H),
        "rob2": ro_b2.reshape(H, 1),
        "row3": ro_w3,
        "ident": np.eye(P, dtype=np.float32),
    }
    in_maps = []
    for c in range(NC):
        m = dict(shared)
        m["ea_b"] = pca[c]["ea_b"]
        m["ind_b"] = pca[c]["ind_b"]
        m["idx_w"] = pca[c]["idx_w"]
        m["zq"] = pca[c]["zq"]
        m["gind"] = pca[c]["gind"]
        in_maps.append(m)

    res = bass_utils.run_bass_kernel_spmd(nc, in_maps, core_ids=list(range(NC)))

    import os as _os
    if _os.environ.get("KDBG"):
        np.save("/tmp/hdump.npy", res.results[0]["hdump"])
        np.save("/tmp/nbound.npy", meta["n_bound"])

    g_bound = meta["g_bound"]
    out = np.zeros((G, 1), np.float32)
    for c in range(NC):
        gs, ge = g_bound[c], g_bound[c + 1]
        out[gs:ge] = res.results[c]["out"][: ge - gs]
    return out
